# revision 1
# baseline (speedup 1.0000x reference)
"""MLA/MQA attention (nn_Attention_33406255628587) on 8 TRN2 cores, v2.

Sharding: batch x4, query-halves x2.  Core (2b+s) handles batch b and
query blocks {s, s+2} (L=block s, H=block s+2) -- balanced causal load,
one uniform SPMD program; per-core causality lives in additive mask DATA
(key>query formula) applied to the L pairs and the last two H pairs.

Dtypes (validated numerically, rel err ~6e-3 vs 2e-2 gate):
  - q path fp8e4 DoubleRow (weights host-scaled x32; latents stored
    pre-normalized: rsbq multiplied in right after the down-proj)
  - kv chain bf16 (down/up, unnormalized latents; 1/rms folded into the
    kf rope tables per-column and into the v copy via per-partition scale)
  - scores always fp8 via u.w trick: scores = [q; q_pe].[a*kf; (1-a)*kf]
    (256-deep contraction -> DoubleRow)
  - attention po/pr: far pairs (H pairs 0-1) fp8 DoubleRow; own-block/near
    pairs (L pairs, H pairs 2-3) bf16 es+v  (early queries need precision)
  - o_proj bf16

Schedule: q-down (dk-outer, streams weight chunks) -> kv-down; per-head
q-up + L-attention software-pipelined (scores(h) emitted before po(h-1));
H-attention pipelined and interleaved with L o_proj tiles; H o_proj last.
PSUM groups: po then pr run as sequential groups in one shared region.
"""

import sys

sys.path.insert(0, "/opt/trn_rl_repo")

import numpy as np

B, T, D, H, HD = 4, 1024, 2048, 16, 128
QR, KVR = 1536, 512
EPS = 1e-6
NEG = -1000000000.0
SCALE = HD ** -0.5
WS = 32.0

P = 128
H2 = 64
TQ = 512          # queries per core
QB = 256          # query block
DKD = D // 256    # 8  dbl chunks of D
QRM = QR // P     # 12
QRD = QR // 256   # 6
KVM = KVR // P    # 4
DK = D // P       # 16
KT = T // P       # 8 key chunks

_nc_cache = {}


def build_kernel(use_pad=False):
    import concourse.bacc as bacc
    import concourse.tile as tile
    from concourse import mybir
    from contextlib import ExitStack

    F32 = mybir.dt.float32
    BF = mybir.dt.bfloat16
    F8 = mybir.dt.float8e4
    AF = mybir.ActivationFunctionType
    DR = mybir.MatmulPerfMode.DoubleRow
    mul = mybir.AluOpType.mult
    add = mybir.AluOpType.add
    sub = mybir.AluOpType.subtract

    nc = bacc.Bacc("TRN2", target_bir_lowering=False, debug=False)

    # ---- DRAM I/O (host-prepared layouts, see _prep_core_inputs) ----
    hq8 = nc.dram_tensor("hq8", [P, DKD, 2, TQ], F8, kind="ExternalInput")
    hk8h = nc.dram_tensor("hk8h", [P, DKD, 2, T], F8, kind="ExternalInput")
    hk8l = nc.dram_tensor("hk8l", [P, DKD, 2, T], F8, kind="ExternalInput")
    wqa8 = nc.dram_tensor("wqa8", [DKD, P, QRM, 2, P], F8, kind="ExternalInput")
    wqb8 = nc.dram_tensor("wqb8", [H, P, QRD, 2, P], F8, kind="ExternalInput")
    wkv8h = nc.dram_tensor("wkv8h", [KVM, P, DKD, 2, P], F8,
                           kind="ExternalInput")
    wkv8l = nc.dram_tensor("wkv8l", [KVM, P, DKD, 2, P], F8,
                           kind="ExternalInput")
    wkvbb = nc.dram_tensor("wkvbb", [P, 2, KVM, P], BF, kind="ExternalInput")
    wob = nc.dram_tensor("wob", [4, P, H, 512], BF, kind="ExternalInput")
    cosq = nc.dram_tensor("cosq", [P, TQ], F32, kind="ExternalInput")
    sinq = nc.dram_tensor("sinq", [P, TQ], F32, kind="ExternalInput")
    c1kv = nc.dram_tensor("c1kv", [P, T], F32, kind="ExternalInput")
    s1kv = nc.dram_tensor("s1kv", [P, T], F32, kind="ExternalInput")
    maskh = nc.dram_tensor("maskh", [P, 4, QB], mybir.dt.bfloat16,
                           kind="ExternalInput")
    gct = nc.dram_tensor("gct", [P, 2], F32, kind="ExternalInput")  # (32a, 32(1-a))
    if use_pad:
        maskp = nc.dram_tensor("maskp", [P, 4, QB], F32, kind="ExternalInput")
    idb = nc.dram_tensor("idb", [P, P], mybir.dt.bfloat16, kind="ExternalInput")
    masklb = nc.dram_tensor("masklb", [P, 4, QB], mybir.dt.bfloat16,
                            kind="ExternalInput")
    o_part = nc.dram_tensor("o_part", [TQ, D], F32, kind="ExternalOutput")

    ESC = SCALE / (WS * WS)   # exp scale: scores psum = 1024 * true scores

    with tile.TileContext(nc, pool_alloc_mode="queue") as tc, ExitStack() as top:
        consts = top.enter_context(tc.tile_pool(name="consts", bufs=1))
        # persistent pools first (LIFO pool discipline: transient pools are
        # created after every long-lived one)
        latp = top.enter_context(tc.tile_pool(name="latp", bufs=1))
        kvnb = latp.tile([P, KVM, T], BF)      # 8KB/p unnormalized kv latents
        qn8 = latp.tile([P, QRD, 2, TQ], F8)   # 6KB/p NORMALIZED q latents
        normp = top.enter_context(tc.tile_pool(name="normp", bufs=1))
        rsbq = normp.tile([P, TQ], F32)        # 1/(32*rms_q), bcast partitions
        rsbkv = normp.tile([P, T], F32)        # 1/rms_kv, bcast partitions
        rsbkv_t = normp.tile([P, KT], F32)     # 1/rms_kv, keys on partitions
        wkvbp = top.enter_context(tc.tile_pool(name="wkvbp", bufs=1))
        wb = wkvbp.tile([P, 2, KVM, P], BF)
        kvp = top.enter_context(tc.tile_pool(name="kvp", bufs=1))
        w8 = kvp.tile([P, 2, T], F8)           # [a*kf; (1-a)*kf] x32
        v_b = kvp.tile([P, KT, P], BF)
        v8 = kvp.tile([P, 2, 2, P], F8)        # key pairs 0-1 (chunks 0-3)

        # --- q-path DMAs first (SP/Act queues): PE starts on q-down ---
        dctx = ExitStack()
        wqap = dctx.enter_context(tc.tile_pool(name="wqap", bufs=1))
        hqp = dctx.enter_context(tc.tile_pool(name="hqp", bufs=1))
        hq_sb = hqp.tile([P, DKD, 2, TQ], F8)  # 8KB/p
        wkvap = dctx.enter_context(tc.tile_pool(name="wkvap", bufs=1))
        hkp = dctx.enter_context(tc.tile_pool(name="hkp", bufs=1))
        hkh_sb = hkp.tile([P, DKD, 2, T], F8)  # 16KB/p
        hkl_sb = hkp.tile([P, DKD, 2, T], F8)  # 16KB/p
        # all input DMAs on one queue (SP) in strict consumption order --
        # the DMA device is a single serial resource, so priority is order
        w_dk, w_kv = [], []
        for dk in range(DKD):
            nc.sync.dma_start(hq_sb[:, dk, :, :], hq8[:, dk, :, :])
            w = wqap.tile([P, QRM, 2, P], F8, tag=f"wqa{dk}", name=f"wqa_{dk}")
            nc.sync.dma_start(w[:], wqa8[dk])
            w_dk.append(w)
            if 2 <= dk < 2 + KVM:   # interleave kv hi-weights into stream
                m_ = dk - 2
                wvh = wkvap.tile([P, DKD, 2, P], F8, tag=f"wkvh{m_}",
                                 name=f"wkvh_{m_}")
                nc.sync.dma_start(wvh[:], wkv8h[m_])
                w_kv.append(wvh)
        gc_sb = consts.tile([P, 2], F32)
        nc.scalar.dma_start(gc_sb[:], gct[:])
        id_sb = consts.tile([P, P], BF)
        nc.scalar.dma_start(id_sb[:], idb[:])
        for dk in range(DKD):
            nc.sync.dma_start(hkh_sb[:, dk, :, :], hk8h[:, dk, :, :])
        # lo residual tensors last: only needed 2/3 into each kv-down group
        w_kvl = []
        for m_ in range(KVM):
            wvl = wkvap.tile([P, DKD, 2, P], F8, tag=f"wkvl{m_}",
                             name=f"wkvl_{m_}")
            nc.sync.dma_start(wvl[:], wkv8l[m_])
            w_kvl.append(wvl)
        for dk in range(DKD):
            nc.sync.dma_start(hkl_sb[:, dk, :, :], hk8l[:, dk, :, :])
        w_kv = [(w_kv[i], w_kvl[i]) for i in range(KVM)]
        nc.sync.dma_start(wb[:], wkvbb[:])
        cq_t = consts.tile([P, TQ], F32)       # raw blended cos/sin for q
        sq_t = consts.tile([P, TQ], F32)
        ck_raw = consts.tile([P, T], F32)
        sk_raw = consts.tile([P, T], F32)
        nc.sync.dma_start(ck_raw[:], c1kv[:])
        nc.sync.dma_start(sk_raw[:], s1kv[:])
        nc.sync.dma_start(cq_t[:], cosq[:])
        nc.sync.dma_start(sq_t[:], sinq[:])
        ml_sb = consts.tile([P, 4, QB], BF)
        mh_sb = consts.tile([P, 4, QB], BF)
        nc.sync.dma_start(ml_sb[:], masklb[:])
        nc.sync.dma_start(mh_sb[:], maskh[:])
        if use_pad:
            mp_sb = consts.tile([P, 4, QB], F32)
            nc.sync.dma_start(mp_sb[:], maskp[:])
        ones8w = consts.tile([P, 2, P], F8)
        nc.vector.memset(ones8w[:], 1.0)
        ones8 = ones8w[:, :, 0:1]
        onesb = consts.tile([P, 2], BF)
        nc.vector.memset(onesb[:], 1.0)
        eps_sb = consts.tile([P, 1], F32)
        nc.vector.memset(eps_sb[:], 1024.0 * EPS)
        epsn_sb = consts.tile([P, 1], F32)
        nc.vector.memset(epsn_sb[:], EPS)

        # ---------- q down-proj (fp8 DoubleRow, dk-outer streaming) ----------
        # Two halves of 6 m-chunks so the 6 live psums (+ss) fit in 8 banks;
        # dk-outer order lets the PE consume weight chunks as they stream in.
        with tc.tile_pool(name="sqq", bufs=1) as sqqp, \
             tc.tile_pool(name="qltmp", bufs=1) as qltmp, \
             tc.tile_pool(name="ps_qd", bufs=1, space="PSUM") as psqd, \
             tc.tile_pool(name="ps_ssq", bufs=1, space="PSUM") as psssq:
            ss_q = psssq.tile([P, TQ], F32)
            sq_m = [sqqp.tile([P, 2, TQ], F8, tag=f"sqq{dm}",
                              name=f"sqq_{dm}") for dm in range(QRD)]
            # q_lat parked in SBUF f32 so psums free up per half
            qlat = qltmp.tile([P, QRM, TQ], F32)   # 24KB/p, freed after qn8
            for half in range(2):
                ms = list(range(6 * half, 6 * half + 6))
                ps_m = {m: psqd.tile([P, TQ], F32, tag=f"psqd{m % 6}",
                                     name=f"psqd_{m}") for m in ms}
                for dk in range(DKD):
                    for m in ms:
                        for tq in range(2):
                            ts = slice(tq * 256, (tq + 1) * 256)
                            nc.tensor.matmul(
                                ps_m[m][:, ts], w_dk[dk][:, m, :, :],
                                hq_sb[:, dk, :, ts],
                                start=(dk == 0 and tq == 0),
                                stop=(dk == DKD - 1 and tq == 1),
                                perf_mode=DR)
                for m in ms:
                    nc.scalar.activation(sq_m[m // 2][:, m % 2, :],
                                         ps_m[m][:], AF.Square,
                                         scale=1.0 / WS)
                    nc.vector.tensor_copy(qlat[:, m, :], ps_m[m][:])
            # ss_q: one 2KB region, single merged group
            for dm in range(QRD):
                for tq in range(2):
                    ts = slice(tq * 256, (tq + 1) * 256)
                    nc.tensor.matmul(
                        ss_q[:, ts], ones8w[:], sq_m[dm][:, :, ts],
                        start=(dm == 0 and tq == 0),
                        stop=(dm == QRD - 1 and tq == 1), perf_mode=DR)
            nc.scalar.activation(rsbq[:], ss_q[:], AF.Sqrt,
                                 bias=eps_sb[:], scale=1024.0 / QR)
            nc.vector.reciprocal(rsbq[:], rsbq[:])
            # qn8 = q_lat * rsbq  (pre-normalized latents)
            for m in range(QRM):
                nc.vector.tensor_tensor(qn8[:, m // 2, m % 2, :],
                                        qlat[:, m, :], rsbq[:], mul)

        # ---------- kv down-proj (bf16) + up-proj, overlapped ----------
        with tc.tile_pool(name="sqkv", bufs=2) as sqkvp, \
             tc.tile_pool(name="kfp", bufs=1) as kfp, \
             tc.tile_pool(name="rtmp", bufs=2) as rtmp, \
             tc.tile_pool(name="ps_sskv", bufs=1, space="PSUM") as pssskv, \
             tc.tile_pool(name="ps_sst", bufs=1, space="PSUM") as psst:
            ss_kv = pssskv.tile([P, 2, 512], F32)
            ss_t = psst.tile([P, KT], F32)
            sq_kv = [sqkvp.tile([P, 2, T], F8, tag=f"sqkv{dm}",
                                name=f"sqkv_{dm}") for dm in range(2)]
            kf = kfp.tile([P, T], F32)
            with tc.tile_pool(name="ps_kvd", bufs=2, space="PSUM") as pskvd:
                for m in range(KVM):
                    wvh, wvl = w_kv[m]
                    ps = pskvd.tile([P, T], F32, tag="pskvd")
                    for tn in range(2):   # one merged group per 2KB bank
                        for tq in range(2):
                            ts = slice(tn * 512 + tq * 256,
                                       tn * 512 + (tq + 1) * 256)
                            k = 0
                            for hs_, ws_ in ((hkh_sb, wvh), (hkl_sb, wvh),
                                             (hkh_sb, wvl)):
                                for dk in range(DKD):
                                    nc.tensor.matmul(
                                        ps[:, ts], ws_[:, dk, :, :],
                                        hs_[:, dk, :, ts],
                                        start=(tq == 0 and k == 0),
                                        stop=(tq == 1
                                              and k == 3 * DKD - 1),
                                        perf_mode=DR)
                                    k += 1
                    nc.scalar.copy(kvnb[:, m, :], ps[:])
                    nc.scalar.activation(sq_kv[m // 2][:, m % 2, :], ps[:],
                                         AF.Square, scale=1.0 / WS)
            # kv up-proj matmuls next: PE overlaps the Act/DVE norm chain
            with tc.tile_pool(name="ps_k", bufs=2, space="PSUM") as psk, \
                 tc.tile_pool(name="ps_v", bufs=1, space="PSUM") as psv:
                ps_vt = psv.tile([P, KT, P], F32)   # all 8 v chunks, 2 banks
                for tn in range(2):
                    ts = slice(tn * 512, (tn + 1) * 512)
                    ps = psk.tile([P, 512], F32, tag="psk")
                    for m in range(KVM):
                        nc.tensor.matmul(ps[:], wb[:, 0, m, :],
                                         kvnb[:, m, ts],
                                         start=(m == 0), stop=(m == KVM - 1))
                    # kf = rope_gate(ps); raw tables (rsbkv folded later)
                    ta = rtmp.tile([P, 512], F32, tag="ta")
                    tb = rtmp.tile([P, 512], F32, tag="tb")
                    nc.vector.tensor_tensor(ta[:], ps[:], ck_raw[:, ts], mul)
                    nc.vector.tensor_tensor(tb[0:H2, :], ps[H2:P, :],
                                            sk_raw[0:H2, ts], mul)
                    nc.vector.tensor_tensor(tb[H2:P, :], ps[0:H2, :],
                                            sk_raw[H2:P, ts], mul)
                    nc.vector.tensor_tensor(kf[0:H2, ts], ta[0:H2, :],
                                            tb[0:H2, :], sub)
                    nc.vector.tensor_tensor(kf[H2:P, ts], ta[H2:P, :],
                                            tb[H2:P, :], add)
                for kc in range(KT):
                    ks = slice(kc * P, (kc + 1) * P)
                    for m in range(KVM):
                        nc.tensor.matmul(ps_vt[:, kc, :], kvnb[:, m, ks],
                                         wb[:, 1, m, :],
                                         start=(m == 0), stop=(m == KVM - 1))
                # ss matmuls + norm chain (Act squares done by now)
                for dm in range(2):
                    for tq in range(4):
                        ts = slice(tq * 256, (tq + 1) * 256)
                        tnn, to = tq // 2, (tq % 2) * 256
                        nc.tensor.matmul(
                            ss_kv[:, tnn, to:to + 256], ones8w[:],
                            sq_kv[dm][:, :, ts],
                            start=(dm == 0 and to == 0),
                            stop=(dm == 1 and to == 256), perf_mode=DR)
                    for kc in range(KT):
                        ks = slice(kc * P, (kc + 1) * P)
                        nc.tensor.matmul(
                            ss_t[:, kc:kc + 1], sq_kv[dm][:, :, ks],
                            ones8[:],
                            start=(dm == 0 and kc == 0),
                            stop=(dm == 1 and kc == KT - 1), perf_mode=DR)
                for tn in range(2):
                    nc.scalar.activation(rsbkv[:, tn * 512:(tn + 1) * 512],
                                         ss_kv[:, tn, :], AF.Sqrt,
                                         bias=eps_sb[:], scale=1024.0 / KVR)
                nc.vector.reciprocal(rsbkv[:], rsbkv[:])
                nc.scalar.activation(rsbkv_t[:], ss_t[:], AF.Sqrt,
                                     bias=eps_sb[:], scale=1024.0 / KVR)
                nc.vector.reciprocal(rsbkv_t[:], rsbkv_t[:])
                for kc in range(KT):
                    nc.scalar.activation(v_b[:, kc, :], ps_vt[:, kc, :],
                                         AF.Identity,
                                         scale=rsbkv_t[:, kc:kc + 1])
                for pc in range(2):
                    nc.scalar.copy(v8[:, pc, :, :],
                                   v_b[:, 2 * pc:2 * pc + 2, :])
                # fold rsbkv once, then w8 = [32a*kf ; 32(1-a)*kf]
                nc.vector.tensor_tensor(kf[:], kf[:], rsbkv[:], mul)
                nc.vector.tensor_scalar(out=w8[:, 0, :], in0=kf[:],
                                        scalar1=gc_sb[:, 0:1], scalar2=None,
                                        op0=mul)
                nc.vector.tensor_scalar(out=w8[:, 1, :], in0=kf[:],
                                        scalar1=gc_sb[:, 1:2], scalar2=None,
                                        op0=mul)
        dctx.close()

        # ---------- q up-proj + attention + o_proj (sw-pipelined) ----------
        qfp = top.enter_context(tc.tile_pool(name="qfp", bufs=1))
        u8 = qfp.tile([P, H, 2, TQ], F8)       # 16KB/p
        outp = top.enter_context(tc.tile_pool(name="outp", bufs=1))
        outT = outp.tile([P, H, TQ], BF)       # 16KB/p
        wop = top.enter_context(tc.tile_pool(name="wop", bufs=1))

        def q_up(h, w_h, rtmp, psqu):
            ps = psqu.tile([P, TQ], F32, tag="psqu")
            for tq in range(2):
                ts = slice(tq * 256, (tq + 1) * 256)
                for dm in range(QRD):
                    nc.tensor.matmul(ps[:, ts], w_h[:, dm, :, :],
                                     qn8[:, dm, :, ts],
                                     start=(dm == 0), stop=(dm == QRD - 1),
                                     perf_mode=DR)
            # qn8 pre-normalized: u0 is a plain copy (Act).  Rope: ta/tb
            # produced bf16 (f32-in, convert-on-write); sinq's lower table
            # half is host-negated so the combine is ONE full-width bf16
            # subtract (2x DVE); Act casts the result to fp8.
            nc.scalar.copy(u8[:, h, 0, :], ps[:])
            ta = rtmp.tile([P, TQ], BF, tag="ta")
            tb = rtmp.tile([P, TQ], BF, tag="tb")
            u1b = rtmp.tile([P, TQ], BF, tag="u1b")
            nc.vector.tensor_tensor(ta[:], ps[:], cq_t[:], mul)
            nc.vector.tensor_tensor(tb[0:H2, :], ps[H2:P, :],
                                    sq_t[0:H2, :], mul)
            nc.vector.tensor_tensor(tb[H2:P, :], ps[0:H2, :],
                                    sq_t[H2:P, :], mul)
            nc.vector.tensor_tensor(u1b[:], ta[:], tb[:], sub)
            nc.scalar.copy(u8[:, h, 1, :], u1b[:])

        def attn_scores(blk, h, expp, ps_s):
            """Emit scores+mask+exp for head h; returns es tiles."""
            qs = slice(blk * QB, (blk + 1) * QB)
            msk = mh_sb if blk else ml_sb
            es8 = None
            if blk:  # far pairs 0-1, fp8
                pss = ps_s.tile([P, 4, QB], F32, tag="pss")
                for kc in range(4):
                    nc.tensor.matmul(pss[:, kc, :],
                                     w8[:, :, kc * P:(kc + 1) * P],
                                     u8[:, h, :, qs], start=True,
                                     stop=True, perf_mode=DR)
                if use_pad:
                    nc.vector.tensor_tensor(pss[:], pss[:], mp_sb[:], add)
                es8 = expp.tile([P, 4, QB], F8, tag="es8")
                nc.scalar.activation(es8[:], pss[:], AF.Exp, bias=0.0,
                                     scale=ESC)
            k0 = 4 if blk else 0
            pss = ps_s.tile([P, 4, QB], F32, tag="pss")
            for j in range(4):
                kc = k0 + j
                nc.tensor.matmul(pss[:, j, :],
                                 w8[:, :, kc * P:(kc + 1) * P],
                                 u8[:, h, :, qs], start=True,
                                 stop=blk == 1, perf_mode=DR)
                if blk == 0:   # causal mask folded in on the PE itself
                    nc.tensor.matmul(pss[:, j, :], id_sb[:],
                                     ml_sb[:, j, :], start=False, stop=True)
            esb = expp.tile([P, 4, QB], BF, tag="esb")
            nc.scalar.activation(esb[:], pss[:], AF.Exp, bias=0.0,
                                 scale=ESC)
            if blk:
                # causal mask as post-exp 0/1 multiply: full-width bf16 2x.
                # unmasked scores are bounded (|s*ESC| < ~3), so exp cannot
                # overflow before the zeroing multiply.
                nc.vector.tensor_tensor(esb[:], esb[:], msk[:], mul)
            return es8, esb

        def attn_po(blk, h, es, atmp, ps_o):
            """po group then pr group (sequential groups, shared region)."""
            qs = slice(blk * QB, (blk + 1) * QB)
            es8, esb = es
            k0 = 4 if blk else 0
            po_t = ps_o.tile([P, 2, QB], F32, tag="po")
            po = po_t[:, 0, :]
            pr = po_t[0:1, 1, :]
            if blk:
                for pc in range(2):
                    nc.tensor.matmul(po[:], v8[:, pc, :, :],
                                     es8[:, 2 * pc:2 * pc + 2, :],
                                     start=(pc == 0), stop=False,
                                     perf_mode=DR)
            for j in range(4):
                nc.tensor.matmul(po[:], v_b[:, k0 + j, :], esb[:, j, :],
                                 start=(not blk and j == 0), stop=(j == 3))
            if blk:
                for pc in range(2):
                    nc.tensor.matmul(pr[:], ones8[:],
                                     es8[:, 2 * pc:2 * pc + 2, :],
                                     start=(pc == 0), stop=False,
                                     perf_mode=DR)
            for j in range(4):
                nc.tensor.matmul(pr[:], onesb[:, 0:1], esb[:, j, :],
                                 start=(not blk and j == 0), stop=(j == 3))
            r1r = atmp.tile([1, QB], F32, tag="r1r")
            nc.vector.reciprocal(r1r[:], pr[:])
            rb = atmp.tile([P, QB], F32, tag="rb")
            nc.gpsimd.partition_broadcast(rb[:], r1r[:])
            nc.vector.tensor_tensor(outT[:, h, qs], po[:], rb[:], mul)

        def oproj_tile(i, w_nts, psw, osb, blk):
            qt = blk * 2 + i // 4
            nt = i % 4
            ps = psw.tile([P, 512], F32, tag="psw")
            for h in range(H):
                nc.tensor.matmul(
                    ps[:], outT[:, h, qt * P:(qt + 1) * P],
                    w_nts[nt][:, h, :],
                    start=(h == 0), stop=(h == H - 1))
            ot = osb.tile([P, 512], F32, tag="ot")
            nc.vector.tensor_copy(ot[:], ps[:])
            nc.sync.dma_start(
                o_part[qt * P:(qt + 1) * P, nt * 512:(nt + 1) * 512],
                ot[:])

        with tc.tile_pool(name="expp", bufs=6) as expp, \
             tc.tile_pool(name="atmp", bufs=4) as atmp, \
             tc.tile_pool(name="ps_s", bufs=2, space="PSUM") as ps_s, \
             tc.tile_pool(name="ps_o", bufs=2, space="PSUM") as ps_o:
            # L phase: q-up + L attention, software-pipelined depth 1
            with tc.tile_pool(name="wqbp", bufs=1) as wqbp, \
                 tc.tile_pool(name="rtmp", bufs=3) as rtmp, \
                 tc.tile_pool(name="ps_qu", bufs=2, space="PSUM") as psqu:
                w_hs = []
                for h in range(H):
                    w_h = wqbp.tile([P, QRD, 2, P], F8, tag=f"wqb{h}",
                                    name=f"wqb_{h}")
                    nc.sync.dma_start(w_h[:], wqb8[h])
                    w_hs.append(w_h)
                w_nts = []
                for nt in range(4):
                    w_nt = wop.tile([P, H, 512], BF, tag=f"wo{nt}",
                                    name=f"wo_{nt}")
                    nc.sync.dma_start(w_nt[:], wob[nt])
                    w_nts.append(w_nt)
                for h in range(5):   # prefetch: fill the w8-wait window
                    q_up(h, w_hs[h], rtmp, psqu)
                prev = None
                for h in range(H):
                    if h + 5 < H:
                        q_up(h + 5, w_hs[h + 5], rtmp, psqu)
                    es = attn_scores(0, h, expp, ps_s)
                    if prev is not None:
                        attn_po(0, prev[0], prev[1], atmp, ps_o)
                    prev = (h, es)
                attn_po(0, prev[0], prev[1], atmp, ps_o)
            # H phase interleaved with L o_proj tiles, pipelined
            with tc.tile_pool(name="ps_w", bufs=2, space="PSUM") as ps_w, \
                 tc.tile_pool(name="osb", bufs=3) as osb:
                prev = None
                for h in range(H):
                    es = attn_scores(1, h, expp, ps_s)
                    if prev is not None:
                        attn_po(1, prev[0], prev[1], atmp, ps_o)
                    prev = (h, es)
                    if h % 2 == 1:
                        oproj_tile(h // 2, w_nts, ps_w, osb, 0)
                attn_po(1, prev[0], prev[1], atmp, ps_o)
                for i in range(8):
                    oproj_tile(i, w_nts, ps_w, osb, 1)

    nc.finalize()
    return nc


def _prep_core_inputs(inputs):
    """Shard + lay out the full inputs for the 8 cores."""
    import ml_dtypes
    F8 = ml_dtypes.float8_e4m3
    BF = ml_dtypes.bfloat16
    f32 = np.float32

    hs = np.asarray(inputs["hidden_states"], f32)
    w_qa = np.asarray(inputs["w_qa"], f32)
    w_qb = np.asarray(inputs["w_qb"], f32)
    w_kva = np.asarray(inputs["w_kva"], f32)
    w_kvb = np.asarray(inputs["w_kvb"], f32)
    qn_w = np.asarray(inputs["qn_w"], f32)
    kvn_w = np.asarray(inputs["kvn_w"], f32)
    w_o = np.asarray(inputs["w_o"], f32)
    att_mask = np.asarray(inputs["attention_mask"])
    for bname in ("b_qa", "b_qb", "b_kva", "b_kvb"):
        assert not np.asarray(inputs[bname], f32).any(), \
            "nonzero projection biases not supported"

    a = float(1.0 / (1.0 + np.exp(-f32(inputs["nope_logit"]))))
    g = float(1.0 / (1.0 + np.exp(-f32(inputs["rope_logit"]))))

    w_qb_f = qn_w[:, None] * w_qb
    w_kvb_f = kvn_w[:, None] * w_kvb

    wqa8 = np.ascontiguousarray(
        (w_qa * WS).reshape(DKD, 2, P, QRM, P).transpose(0, 2, 3, 1, 4)
    ).astype(F8)
    wqb8 = np.ascontiguousarray(
        (w_qb_f * WS).reshape(QRD, 2, P, H, P).transpose(3, 2, 0, 1, 4)
    ).astype(F8)
    wkva32 = w_kva * WS
    wkva_hi = wkva32.astype(F8)
    wkva_lo = (wkva32 - wkva_hi.astype(f32)).astype(F8)
    _wkl = lambda w: np.ascontiguousarray(
        w.reshape(DKD, 2, P, KVM, P).transpose(3, 2, 0, 1, 4)).astype(F8)
    wkv8h = _wkl(wkva_hi.astype(f32))
    wkv8l = _wkl(wkva_lo.astype(f32))
    wkvbb = np.ascontiguousarray(
        w_kvb_f.reshape(KVM, P, 2, P).transpose(1, 2, 0, 3)).astype(BF)
    wob = np.ascontiguousarray(
        w_o.reshape(H, P, 4, 512).transpose(2, 1, 0, 3)).astype(BF)
    gct = np.broadcast_to(
        np.array([WS * a, WS * (1.0 - a)], f32), (P, 2)).copy()

    cosb = g * np.asarray(inputs["cos_g"], f32) + (1 - g) * np.asarray(inputs["cos_l"], f32)
    sinb = g * np.asarray(inputs["sin_g"], f32) + (1 - g) * np.asarray(inputs["sin_l"], f32)

    i_p = np.arange(P)
    i_q = np.arange(QB)
    in_maps = []
    for c in range(NCORES):
        b, s = c // 2, c % 2
        blocks = [s, s + 2]
        qcols = np.concatenate([np.arange(bb * QB, (bb + 1) * QB)
                                for bb in blocks])
        hq8 = np.ascontiguousarray(
            hs[b][qcols, :].T.reshape(DKD, 2, P, TQ).transpose(2, 0, 1, 3)
        ).astype(F8)
        hkT = hs[b].T.reshape(DKD, 2, P, T).transpose(2, 0, 1, 3)
        hk_hi = np.ascontiguousarray(hkT).astype(F8)
        hk_lo = np.ascontiguousarray(
            hkT - hk_hi.astype(f32)).astype(F8)
        cb_q = cosb[b][qcols, :].T          # [64, TQ]
        sb_q = sinb[b][qcols, :].T
        cosq = np.ascontiguousarray(np.concatenate([cb_q, cb_q], 0))
        sinq = np.ascontiguousarray(np.concatenate([sb_q, -sb_q], 0))
        cb_k = cosb[b].T                    # [64, T]
        sb_k = sinb[b].T
        c1 = a + (1 - a) * cb_k
        s1 = (1 - a) * sb_k
        c1kv = np.ascontiguousarray(np.concatenate([c1, c1], 0))
        s1kv = np.ascontiguousarray(np.concatenate([s1, s1], 0))
        # masks: key > query  (+ padding), [P, 4(pair*2+j), QB]
        pad_b = (att_mask[b] == 0)
        use_pad = bool(pad_b.any())
        masks = []
        for mi, blk in enumerate(blocks):
            koff = mi * 512  # L-mask covers keys [0:512), H-mask [512:1024)
            key_abs = koff + (np.arange(4)[:, None, None] * P
                              + i_p[None, :, None])        # [4, P, 1]
            q_abs = blk * QB + i_q[None, None, :]          # [1, 1, QB]
            bad = (key_abs > q_abs) | pad_b[key_abs]
            if mi == 0:   # L: additive NEG mask (PE-accumulated)
                m = np.where(bad, NEG, 0.0)
            else:         # H: post-exp 0/1 multiplier
                m = np.where(bad, 0.0, 1.0) + 0.0 * q_abs
            masks.append(np.ascontiguousarray(
                m.transpose(1, 0, 2)).astype(f32))
        imap = {
            "hq8": hq8, "hk8h": hk_hi, "hk8l": hk_lo,
            "wqa8": wqa8, "wqb8": wqb8,
            "wkv8h": wkv8h, "wkv8l": wkv8l, "wkvbb": wkvbb, "wob": wob,
            "cosq": cosq, "sinq": sinq, "c1kv": c1kv, "s1kv": s1kv,
            "masklb": masks[0].astype(BF), "maskh": masks[1].astype(BF),
            "gct": gct,
            "idb": np.eye(P, dtype=np.float32).astype(BF),
        }
        if use_pad:
            # pad-only mask for the H-block far pairs (keys 0-511)
            key_far = (np.arange(4)[:, None, None] * P
                       + i_p[None, :, None])
            mp = np.where(pad_b[key_far], NEG, 0.0) + np.zeros((1, 1, QB))
            imap["maskp"] = np.ascontiguousarray(
                mp.transpose(1, 0, 2)).astype(f32)
        in_maps.append(imap)
    return in_maps


NCORES = 8


def kernel(**inputs):
    use_pad = bool((np.asarray(inputs["attention_mask"]) == 0).any())
    if use_pad not in _nc_cache:
        _nc_cache[use_pad] = build_kernel(use_pad=use_pad)
    nc = _nc_cache[use_pad]

    from concourse.bass_utils import run_bass_kernel_spmd
    in_maps = _prep_core_inputs(inputs)
    res = run_bass_kernel_spmd(nc, in_maps, core_ids=list(range(NCORES)))
    out = np.empty((B, T, D), np.float32)
    for b in range(B):
        for s in range(2):
            r = res.results[2 * b + s]["o_part"]
            for i, blk in enumerate([s, s + 2]):
                out[b, blk * QB:(blk + 1) * QB] = r[i * QB:(i + 1) * QB]
    return out



# revision 25
# speedup vs baseline: 1.0763x; 1.0763x over previous
"""MLA/MQA attention (nn_Attention_33406255628587) on 8 TRN2 cores, v3.

Sharding: batch x4, query-halves x2.  Core (2b+s) handles batch b and
query blocks {s, s+2} (L=block s, H=block s+2) -- balanced causal load,
one uniform SPMD program; per-core causality lives in additive mask DATA
(key>query formula) applied to the L pairs and the last two H pairs.

Dtypes (validated numerically vs the 2e-2 gate):
  - q path fp8e4 DoubleRow (weights host-scaled x32; latents stored
    pre-normalized: rsbq multiplied in right after the down-proj)
  - kv chain bf16 (down/up, unnormalized latents; 1/rms folded into the
    kf rope tables per-column and into the v copy via per-partition scale)
  - scores always fp8 via u.w trick: scores = [q; q_pe].[a*kf; (1-a)*kf]
    (256-deep contraction -> DoubleRow)
  - attention po/pr: far pairs (H pairs 0-1) fp8 DoubleRow; own-block/near
    pairs (L pairs, H pairs 2-3) bf16 es+v  (early queries need precision)
  - o_proj bf16

v3 schedule (PE-idle driven rewrite of v2):
  qdown (dk-outer streams weights) -> q-up all 16 heads (wqb streamed
  right behind the wqa DMAs; rope/u8 conversion lags on DVE/Act into the
  next phase) -> kv-down in m-pair groups with hi/lo STREAM split (hi
  products first, lo-residual products after, so the lo DMAs can trail)
  -> kv-up: ss matmuls + rsqrt chain first, then k-up tn0 -> rope tn0 ->
  w8 tn0 (L attention + H far pairs only need keys 0-511, so L starts
  while the tn1 chain still runs), v-up + v copies between -> L
  attention (pure, sw-pipelined) -> H attention with L o_proj tiles
  interleaved -> H o_proj, last tile split in halves to shorten the
  drain.  kvnb psum->sbuf copies run on DVE (idle there), squares on
  Act, reciprocals emitted before the rope ops so the v/norm chain
  doesn't serialize behind them.
"""

import sys

sys.path.insert(0, "/opt/trn_rl_repo")

import numpy as np

B, T, D, H, HD = 4, 1024, 2048, 16, 128
QR, KVR = 1536, 512
EPS = 1e-6
NEG = -1000000000.0
SCALE = HD ** -0.5
WS = 32.0

P = 128
H2 = 64
TQ = 512          # queries per core
QB = 256          # query block
DKD = D // 256    # 8  dbl chunks of D
QRM = QR // P     # 12
QRD = QR // 256   # 6
KVM = KVR // P    # 4
DK = D // P       # 16
KT = T // P       # 8 key chunks

_nc_cache = {}

_MARKS = []


def _MARK(nc, label):
    _MARKS.append((label, nc.next_id()))


def build_kernel(use_pad=False):
    import concourse.bacc as bacc
    import concourse.tile as tile
    from concourse import mybir
    from contextlib import ExitStack

    F32 = mybir.dt.float32
    BF = mybir.dt.bfloat16
    F8 = mybir.dt.float8e4
    F8E5 = mybir.dt.float8e5
    AF = mybir.ActivationFunctionType
    DR = mybir.MatmulPerfMode.DoubleRow
    mul = mybir.AluOpType.mult
    add = mybir.AluOpType.add
    sub = mybir.AluOpType.subtract

    nc = bacc.Bacc("TRN2", target_bir_lowering=False, debug=False)

    # ---- DRAM I/O (host-prepared layouts, see _prep_core_inputs) ----
    hq8 = nc.dram_tensor("hq8", [P, DKD, 2, TQ], F8, kind="ExternalInput")
    hk8h = nc.dram_tensor("hk8h", [P, DKD, 2, T], F8, kind="ExternalInput")
    hk8l = nc.dram_tensor("hk8l", [P, DKD, 2, T], F8, kind="ExternalInput")
    wqa8 = nc.dram_tensor("wqa8", [DKD, P, QRM, 2, P], F8, kind="ExternalInput")
    wqb8 = nc.dram_tensor("wqb8", [H, P, QRD, 2, P], F8, kind="ExternalInput")
    wkv8h = nc.dram_tensor("wkv8h", [KVM, P, DKD, 2, P], F8,
                           kind="ExternalInput")
    wkv8l = nc.dram_tensor("wkv8l", [KVM, P, DKD, 2, P], F8,
                           kind="ExternalInput")
    wkvbb = nc.dram_tensor("wkvbb", [P, 2, KVM, P], BF, kind="ExternalInput")
    wob = nc.dram_tensor("wob", [4, P, H, 512], BF, kind="ExternalInput")
    cosq = nc.dram_tensor("cosq", [P, TQ], BF, kind="ExternalInput")
    sinq = nc.dram_tensor("sinq", [P, TQ], BF, kind="ExternalInput")
    c1kv = nc.dram_tensor("c1kv", [P, T], F32, kind="ExternalInput")
    s1kv = nc.dram_tensor("s1kv", [P, T], F32, kind="ExternalInput")
    mask8h = nc.dram_tensor("mask8h", [P, 2, 4, QB], F8E5,
                            kind="ExternalInput")
    gct = nc.dram_tensor("gct", [P, 2], F32, kind="ExternalInput")  # (32a, 32(1-a))
    if use_pad:
        maskp = nc.dram_tensor("maskp", [P, 4, QB], F32, kind="ExternalInput")
    id448 = nc.dram_tensor("id448", [P, 2, P], F8E5, kind="ExternalInput")
    mask8l = nc.dram_tensor("mask8l", [P, 2, 4, QB], F8E5,
                            kind="ExternalInput")
    o_part = nc.dram_tensor("o_part", [TQ, D], F32, kind="ExternalOutput")

    ESC = SCALE / (WS * WS)   # exp scale: scores psum = 1024 * true scores

    with tile.TileContext(nc, pool_alloc_mode="queue") as tc, ExitStack() as top:
        consts = top.enter_context(tc.tile_pool(name="consts", bufs=1))
        # persistent pools first (LIFO pool discipline: transient pools are
        # created after every long-lived one)
        latp = top.enter_context(tc.tile_pool(name="latp", bufs=1))
        kvnb = latp.tile([P, KVM, T], BF)      # 8KB/p unnormalized kv latents
        qn8 = latp.tile([P, QRD, 2, TQ], F8)   # 6KB/p NORMALIZED q latents
        normp = top.enter_context(tc.tile_pool(name="normp", bufs=1))
        rsbq = normp.tile([P, TQ], F32)        # 1/(32*rms_q), bcast partitions
        rsbkv = normp.tile([P, T], F32)        # 1/rms_kv, bcast partitions
        rsbkv_t = normp.tile([P, KT], F32)     # 1/rms_kv, keys on partitions
        wkvbp = top.enter_context(tc.tile_pool(name="wkvbp", bufs=1))
        wb = wkvbp.tile([P, 2, KVM, P], BF)
        kvp = top.enter_context(tc.tile_pool(name="kvp", bufs=1))
        w8 = kvp.tile([P, 2, T], F8)           # [a*kf; (1-a)*kf] x32
        v_b = kvp.tile([P, 4, P], BF)          # bf16 v, key chunks 0-3 (L po)
        v8h = kvp.tile([P, 4, 2, P], F8)       # fp8 v, all 8 chunks (H po)
        # q-up weights: pool reserved up-front (24KB/p) so their DMAs can
        # stream right behind the q-down weights
        wqbp = top.enter_context(tc.tile_pool(name="wqbp", bufs=1))
        qfp = top.enter_context(tc.tile_pool(name="qfp", bufs=1))
        u8 = qfp.tile([P, H, 2, TQ], F8)       # 16KB/p
        outp = top.enter_context(tc.tile_pool(name="outp", bufs=1))
        outT = outp.tile([P, H, TQ], BF)       # 16KB/p

        # ---- transient input pools ----
        dctx = ExitStack()
        wkvap = dctx.enter_context(tc.tile_pool(name="wkvap", bufs=1))
        hkp = dctx.enter_context(tc.tile_pool(name="hkp", bufs=1))
        hkh_sb = hkp.tile([P, DKD, 2, T], F8)  # 16KB/p
        hkl_sb = hkp.tile([P, DKD, 2, T], F8)  # 16KB/p
        # q-path pools last so they release (LIFO) right after q-down
        dctx_q = ExitStack()
        wqap = dctx_q.enter_context(tc.tile_pool(name="wqap", bufs=1))
        hqp = dctx_q.enter_context(tc.tile_pool(name="hqp", bufs=1))
        hq_sb = hqp.tile([P, DKD, 2, TQ], F8)  # 8KB/p

        # ---- input DMAs, one serial device, strict consumption order ----
        # qdown stream
        w_dk = []
        for dk in range(DKD):
            nc.sync.dma_start(hq_sb[:, dk, :, :], hq8[:, dk, :, :])
            w = wqap.tile([P, QRM, 2, P], F8, tag=f"wqa{dk}", name=f"wqa_{dk}")
            nc.sync.dma_start(w[:], wqa8[dk])
            w_dk.append(w)
        # q-up needs cos/sin + wqb right after qdown: stream them next
        cq_t = consts.tile([P, TQ], BF)        # raw blended cos/sin for q
        sq_t = consts.tile([P, TQ], BF)
        nc.sync.dma_start(cq_t[:], cosq[:])
        nc.sync.dma_start(sq_t[:], sinq[:])
        w_hs = []
        for h in range(H):
            w_h = wqbp.tile([P, QRD, 2, P], F8, tag=f"wqb{h}",
                            name=f"wqb_{h}")
            nc.sync.dma_start(w_h[:], wqb8[h])
            w_hs.append(w_h)
        gc_sb = consts.tile([P, 2], F32)
        nc.scalar.dma_start(gc_sb[:], gct[:])
        id_sb = consts.tile([P, 2, P], F8E5)   # 448*I in row 0, zeros row 1
        nc.scalar.dma_start(id_sb[:], id448[:])
        # kv-down stream, matching the m-pair hi->lo consumption order:
        # wvh01, hkh, hkl, wvl01, wvh23, wvl23
        w_kvh, w_kvl = [], []
        for m_ in range(2):
            wvh = wkvap.tile([P, DKD, 2, P], F8, tag=f"wkvh{m_}",
                             name=f"wkvh_{m_}")
            nc.sync.dma_start(wvh[:], wkv8h[m_])
            w_kvh.append(wvh)
        for dk in range(DKD):
            nc.sync.dma_start(hkh_sb[:, dk, :, :], hk8h[:, dk, :, :])
        for dk in range(DKD):
            nc.sync.dma_start(hkl_sb[:, dk, :, :], hk8l[:, dk, :, :])
        for m_ in range(2):
            wvl = wkvap.tile([P, DKD, 2, P], F8, tag=f"wkvl{m_}",
                             name=f"wkvl_{m_}")
            nc.sync.dma_start(wvl[:], wkv8l[m_])
            w_kvl.append(wvl)
        for m_ in range(2, KVM):
            wvh = wkvap.tile([P, DKD, 2, P], F8, tag=f"wkvh{m_}",
                             name=f"wkvh_{m_}")
            nc.sync.dma_start(wvh[:], wkv8h[m_])
            w_kvh.append(wvh)
        for m_ in range(2, KVM):
            wvl = wkvap.tile([P, DKD, 2, P], F8, tag=f"wkvl{m_}",
                             name=f"wkvl_{m_}")
            nc.sync.dma_start(wvl[:], wkv8l[m_])
            w_kvl.append(wvl)
        nc.sync.dma_start(wb[:], wkvbb[:])
        ck_raw = consts.tile([P, T], F32)
        sk_raw = consts.tile([P, T], F32)
        nc.sync.dma_start(ck_raw[:], c1kv[:])
        nc.sync.dma_start(sk_raw[:], s1kv[:])
        ml_sb = consts.tile([P, 2, 4, QB], F8E5)
        mh_sb = consts.tile([P, 2, 4, QB], F8E5)
        nc.sync.dma_start(ml_sb[:], mask8l[:])
        nc.sync.dma_start(mh_sb[:], mask8h[:])
        if use_pad:
            mp_sb = consts.tile([P, 4, QB], F32)
            nc.sync.dma_start(mp_sb[:], maskp[:])
        ones8w = consts.tile([P, 2, P], F8)
        nc.vector.memset(ones8w[:], 1.0)
        ones8 = ones8w[:, :, 0:1]
        onesb = consts.tile([P, 2], BF)
        nc.vector.memset(onesb[:], 1.0)
        eps_sb = consts.tile([P, 1], F32)
        nc.vector.memset(eps_sb[:], 1024.0 * EPS)
        epsn_sb = consts.tile([P, 1], F32)
        nc.vector.memset(epsn_sb[:], EPS)

        _MARK(nc, 'qdown')
        # ---------- q down-proj (fp8 DoubleRow, dk-outer streaming) ----------
        # Two halves of 6 m-chunks so the 6 live psums (+ss) fit in 8 banks;
        # dk-outer order lets the PE consume weight chunks as they stream in.
        with tc.tile_pool(name="sqq", bufs=1) as sqqp, \
             tc.tile_pool(name="qltmp", bufs=1) as qltmp, \
             tc.tile_pool(name="ps_qd", bufs=1, space="PSUM") as psqd, \
             tc.tile_pool(name="ps_ssq", bufs=1, space="PSUM") as psssq:
            ss_q = psssq.tile([P, TQ], F32)
            sq_m = [sqqp.tile([P, 2, TQ], F8, tag=f"sqq{dm}",
                              name=f"sqq_{dm}") for dm in range(QRD)]
            # q_lat parked in SBUF bf16 so psums free up per half (the
            # latents end up fp8 in qn8, so bf16 parking costs nothing)
            qlat = qltmp.tile([P, QRM, TQ], BF)    # 12KB/p, freed after qn8
            for half in range(2):
                ms = list(range(6 * half, 6 * half + 6))
                ps_m = {m: psqd.tile([P, TQ], F32, tag=f"psqd{m % 6}",
                                     name=f"psqd_{m}") for m in ms}
                for dk in range(DKD):
                    for m in ms:
                        for tq in range(2):
                            ts = slice(tq * 256, (tq + 1) * 256)
                            nc.tensor.matmul(
                                ps_m[m][:, ts], w_dk[dk][:, m, :, :],
                                hq_sb[:, dk, :, ts],
                                start=(dk == 0 and tq == 0),
                                stop=(dk == DKD - 1 and tq == 1),
                                perf_mode=DR)
                for m in ms:
                    nc.scalar.activation(sq_m[m // 2][:, m % 2, :],
                                         ps_m[m][:], AF.Square,
                                         scale=1.0 / WS)
                    nc.vector.tensor_copy(qlat[:, m, :], ps_m[m][:])
                # this half's ss_q contribution (accumulated across halves)
                for dm in range(3 * half, 3 * half + 3):
                    for tq in range(2):
                        ts = slice(tq * 256, (tq + 1) * 256)
                        nc.tensor.matmul(
                            ss_q[:, ts], ones8w[:], sq_m[dm][:, :, ts],
                            start=(half == 0 and dm == 0 and tq == 0),
                            stop=(half == 1 and dm == 5 and tq == 1),
                            perf_mode=DR)
            nc.scalar.activation(rsbq[:], ss_q[:], AF.Sqrt,
                                 bias=eps_sb[:], scale=1024.0 / QR)
            nc.vector.reciprocal(rsbq[:], rsbq[:])
            # qn8 = q_lat * rsbq  (pre-normalized latents); split DVE/Pool
            # so the serial tail gating q-up is ~4.8us not 7.1us
            for m in range(QRM):
                eng = nc.vector if m < 8 else nc.gpsimd
                eng.tensor_tensor(qn8[:, m // 2, m % 2, :],
                                  qlat[:, m, :], rsbq[:], mul)
        dctx_q.close()

        # ---------- q up-proj (interleaved into kv-down PE stream) ----------
        # Rope per head: Act makes a bf16 psum copy (psb) + the final fp8
        # cast; DVE does the fp8 u0 copy, cos-product, one sin half and the
        # combine (all-bf16 SBUF -> 2x mode); Pool (gpsimd, otherwise idle)
        # takes the other sin half.  The psum frees after the two copies,
        # so the q-up pipeline never waits on the rope tail.
        _MARK(nc, 'qup')

        qup_pend = []   # (h, u1b) whose final Act fp8 cast is deferred one
                        # head, so the Act queue never blocks on the rope

        def qup_flush():
            while qup_pend:
                ph, pu = qup_pend.pop(0)
                nc.scalar.copy(u8[:, ph, 1, :], pu[:])

        def q_up(h, rtmpq, psqu):
            ps = psqu.tile([P, TQ], F32, tag="psqu")
            for tq in range(2):
                ts = slice(tq * 256, (tq + 1) * 256)
                for dm in range(QRD):
                    nc.tensor.matmul(ps[:, ts], w_hs[h][:, dm, :, :],
                                     qn8[:, dm, :, ts],
                                     start=(dm == 0), stop=(dm == QRD - 1),
                                     perf_mode=DR)
            nc.scalar.copy(u8[:, h, 0, :], ps[:])
            qup_flush()
            ta = rtmpq.tile([P, TQ], BF, tag="ta")
            tb = rtmpq.tile([P, TQ], BF, tag="tb")
            u1b = rtmpq.tile([P, TQ], BF, tag="u1b")
            # rope products on DVE straight from the psum (partition-shifted
            # reads are only legal from PSUM); the aligned bf16 combine goes
            # to Pool (gpsimd: SBUF-only, aligned-only)
            nc.vector.tensor_tensor(ta[:], ps[:], cq_t[:], mul)
            nc.vector.tensor_tensor(tb[0:H2, :], ps[H2:P, :],
                                    sq_t[0:H2, :], mul)
            nc.vector.tensor_tensor(tb[H2:P, :], ps[0:H2, :],
                                    sq_t[H2:P, :], mul)
            nc.gpsimd.tensor_tensor(u1b[:], ta[:], tb[:], sub)
            qup_pend.append((h, u1b))

        # ---------- kv down-proj: m-pair groups, hi stream then lo ----------
        with tc.tile_pool(name="sqkv", bufs=2) as sqkvp:
          with tc.tile_pool(name="rtmpq", bufs=4) as rtmpq, \
               tc.tile_pool(name="ps_qu", bufs=2, space="PSUM") as psqu:
            qup_h = iter(range(H))
            for h in (next(qup_h), next(qup_h)):
                q_up(h, rtmpq, psqu)
            _MARK(nc, 'kvdown')
            with tc.tile_pool(name="ps_kvd", bufs=1, space="PSUM") as pskvd:
                sq_kv = [sqkvp.tile([P, 2, T], F8, tag=f"sqkv{dm}",
                                    name=f"sqkv_{dm}") for dm in range(2)]
                unit = 0
                for mp in range(2):
                    mm = (2 * mp, 2 * mp + 1)
                    ps_pair = {m: pskvd.tile([P, T], F32, tag=f"pskvd{m % 2}",
                                             name=f"pskvd_{m}") for m in mm}
                    # hi*hi stream (dk-outer so the PE tracks the hkh DMAs),
                    # then the two lo-residual streams (their DMAs trail);
                    # one q-up head woven in every 3 dk-units
                    for hs_, wgetter, st0, st1 in (
                            (hkh_sb, lambda m: w_kvh[m], True, False),
                            (hkl_sb, lambda m: w_kvh[m], False, False),
                            (hkh_sb, lambda m: w_kvl[m], False, True)):
                        for dk in range(DKD):
                            for m in mm:
                                for sl in range(4):
                                    ts = slice(sl * 256, (sl + 1) * 256)
                                    # psum zero regions are 2KB banks: one
                                    # start/stop per 512-col bank
                                    nc.tensor.matmul(
                                        ps_pair[m][:, ts],
                                        wgetter(m)[:, dk, :, :],
                                        hs_[:, dk, :, ts],
                                        start=(st0 and dk == 0
                                               and sl % 2 == 0),
                                        stop=(st1 and dk == DKD - 1
                                              and sl % 2 == 1),
                                        perf_mode=DR)
                            unit += 1
                            if unit % 3 == 0:
                                h = next(qup_h, None)
                                if h is not None:
                                    q_up(h, rtmpq, psqu)
                    for m in mm:
                        # squares on Act (gate the ss/rsqrt chain), latent
                        # copies on DVE -- tn0 halves first (gate k-tn0/v)
                        nc.scalar.activation(sq_kv[m // 2][:, m % 2, :],
                                             ps_pair[m][:], AF.Square,
                                             scale=1.0 / WS)
                        nc.scalar.copy(kvnb[:, m, 0:512],
                                       ps_pair[m][:, 0:512])
                    for m in mm:
                        nc.scalar.copy(kvnb[:, m, 512:T],
                                       ps_pair[m][:, 512:T])
                for h in qup_h:
                    q_up(h, rtmpq, psqu)
                qup_flush()

          # ---------- kv up-proj + norm chain ----------
          if True:
            _MARK(nc, 'kvup')
            with tc.tile_pool(name="kfp", bufs=1) as kfp, \
                 tc.tile_pool(name="rtmp", bufs=2) as rtmp, \
                 tc.tile_pool(name="ps_sskv", bufs=1, space="PSUM") as pssskv, \
                 tc.tile_pool(name="ps_sst", bufs=1, space="PSUM") as psst, \
                 tc.tile_pool(name="ps_k", bufs=1, space="PSUM") as psk, \
                 tc.tile_pool(name="ps_v", bufs=1, space="PSUM") as psv:
                ss_kv = pssskv.tile([P, 2, 512], F32)
                ss_t = psst.tile([P, KT], F32)
                kf = kfp.tile([P, T], F32)
                ps_vt = psv.tile([P, KT, P], F32)   # all 8 v chunks, 2 banks
                # ss matmuls first: the rsqrt chain runs while the PE does
                # the k/v up-projections
                for dm in range(2):
                    for tq in range(4):
                        ts = slice(tq * 256, (tq + 1) * 256)
                        tnn, to = tq // 2, (tq % 2) * 256
                        nc.tensor.matmul(
                            ss_kv[:, tnn, to:to + 256], ones8w[:],
                            sq_kv[dm][:, :, ts],
                            start=(dm == 0 and to == 0),
                            stop=(dm == 1 and to == 256), perf_mode=DR)
                    for kc in range(KT):
                        ks = slice(kc * P, (kc + 1) * P)
                        nc.tensor.matmul(
                            ss_t[:, kc:kc + 1], sq_kv[dm][:, :, ks],
                            ones8[:],
                            start=(dm == 0 and kc == 0),
                            stop=(dm == 1 and kc == KT - 1), perf_mode=DR)
                for tn in range(2):
                    nc.scalar.activation(rsbkv[:, tn * 512:(tn + 1) * 512],
                                         ss_kv[:, tn, :], AF.Sqrt,
                                         bias=eps_sb[:], scale=1024.0 / KVR)
                nc.scalar.activation(rsbkv_t[:], ss_t[:], AF.Sqrt,
                                     bias=eps_sb[:], scale=1024.0 / KVR)
                # reciprocals BEFORE the rope ops in the DVE queue: tn0
                # first (it gates w8 tn0 and the L phase), then the v scale
                nc.vector.reciprocal(rsbkv[:, 0:512], rsbkv[:, 0:512])
                nc.vector.reciprocal(rsbkv_t[:], rsbkv_t[:])
                nc.vector.reciprocal(rsbkv[:, 512:T], rsbkv[:, 512:T])
                for tn in range(2):
                    ts = slice(tn * 512, (tn + 1) * 512)
                    ps = psk.tile([P, 512], F32, tag="psk")
                    for m in range(KVM):
                        nc.tensor.matmul(ps[:], wb[:, 0, m, :],
                                         kvnb[:, m, ts],
                                         start=(m == 0), stop=(m == KVM - 1))
                    # kf = rope_gate(ps); raw tables (rsbkv folded below)
                    ta = rtmp.tile([P, 512], F32, tag="ta")
                    tb = rtmp.tile([P, 512], F32, tag="tb")
                    nc.vector.tensor_tensor(ta[:], ps[:], ck_raw[:, ts], mul)
                    nc.vector.tensor_tensor(tb[0:H2, :], ps[H2:P, :],
                                            sk_raw[0:H2, ts], mul)
                    nc.vector.tensor_tensor(tb[H2:P, :], ps[0:H2, :],
                                            sk_raw[H2:P, ts], mul)
                    nc.vector.tensor_tensor(kf[0:H2, ts], ta[0:H2, :],
                                            tb[0:H2, :], sub)
                    nc.vector.tensor_tensor(kf[H2:P, ts], ta[H2:P, :],
                                            tb[H2:P, :], add)
                    # fold rsbkv + write both w8 gate rows for this tn so
                    # the L phase (keys 0-511) unblocks after tn=0
                    nc.vector.tensor_tensor(kf[:, ts], kf[:, ts],
                                            rsbkv[:, ts], mul)
                    nc.vector.tensor_scalar(out=w8[:, 0, ts], in0=kf[:, ts],
                                            scalar1=gc_sb[:, 0:1],
                                            scalar2=None, op0=mul)
                    nc.vector.tensor_scalar(out=w8[:, 1, ts], in0=kf[:, ts],
                                            scalar1=gc_sb[:, 1:2],
                                            scalar2=None, op0=mul)
                    # v up-proj for this tn's key chunks right after the
                    # k chain; Act does the per-partition rms scale
                    for kc in range(4 * tn, 4 * tn + 4):
                        ks = slice(kc * P, (kc + 1) * P)
                        for m in range(KVM):
                            nc.tensor.matmul(ps_vt[:, kc, :],
                                             kvnb[:, m, ks],
                                             wb[:, 1, m, :],
                                             start=(m == 0),
                                             stop=(m == KVM - 1))
                    for kc in range(4 * tn, 4 * tn + 4):
                        nc.scalar.activation(v8h[:, kc // 2, kc % 2, :],
                                             ps_vt[:, kc, :],
                                             AF.Identity,
                                             scale=rsbkv_t[:, kc:kc + 1])
                        if tn == 0:
                            nc.scalar.activation(v_b[:, kc, :],
                                                 ps_vt[:, kc, :],
                                                 AF.Identity,
                                                 scale=rsbkv_t[:, kc:kc + 1])
        dctx.close()

        # ---------- attention + o_proj (sw-pipelined) ----------
        _MARK(nc, 'Lphase')
        wop = top.enter_context(tc.tile_pool(name="wop", bufs=1))

        def attn_scores(blk, h, expp, ps_s):
            """Emit scores+mask+exp for head h; returns es tiles.

            Masks are additive on the PE: a DoubleRow inject of the fp8
            {0,-448} mask through the 448*I stationary adds -200704 to the
            psum (exp arg -17.3 -> flushes to exactly 0 in fp8/bf16).
            L keeps bf16 es (early queries: few-key softmax needs the
            precision); H is all-fp8 so po/pr run DoubleRow.
            """
            qs = slice(blk * QB, (blk + 1) * QB)
            es8 = None
            if blk:  # far pairs 0-1, fp8, no causal mask needed
                pss = ps_s.tile([P, 4, QB], F32, tag="pss")
                for kc in range(4):
                    nc.tensor.matmul(pss[:, kc, :],
                                     w8[:, :, kc * P:(kc + 1) * P],
                                     u8[:, h, :, qs], start=True,
                                     stop=True, perf_mode=DR)
                if use_pad:
                    nc.vector.tensor_tensor(pss[:], pss[:], mp_sb[:], add)
                es8 = expp.tile([P, 4, QB], F8, tag="es8")
                nc.scalar.activation(es8[:], pss[:], AF.Exp, bias=0.0,
                                     scale=ESC)
            k0 = 4 if blk else 0
            msk = mh_sb if blk else ml_sb
            pss = ps_s.tile([P, 4, QB], F32, tag="pss")
            for j in range(4):
                kc = k0 + j
                nc.tensor.matmul(pss[:, j, :],
                                 w8[:, :, kc * P:(kc + 1) * P],
                                 u8[:, h, :, qs], start=True,
                                 stop=False, perf_mode=DR)
                nc.tensor.matmul(pss[:, j, :], id_sb[:],
                                 msk[:, :, j, :], start=False, stop=True,
                                 perf_mode=DR)
            esb = expp.tile([P, 4, QB], BF if blk == 0 else F8, tag="esb")
            nc.scalar.activation(esb[:], pss[:], AF.Exp, bias=0.0,
                                 scale=ESC)
            return es8, esb

        def attn_po(blk, h, es, atmp, ps_o):
            """po group then pr group (sequential groups, shared region).
            H (blk=1): everything fp8 DoubleRow (es8 far + fp8 near + v8h).
            L (blk=0): bf16 es x bf16 v (early-query precision)."""
            qs = slice(blk * QB, (blk + 1) * QB)
            es8, esb = es
            po_t = ps_o.tile([P, 2, QB], F32, tag="po")
            po = po_t[:, 0, :]
            pr = po_t[0:1, 1, :]
            if blk:
                for pc, est in ((0, es8), (1, es8), (2, esb), (3, esb)):
                    nc.tensor.matmul(po[:], v8h[:, pc, :, :],
                                     est[:, 2 * (pc % 2):2 * (pc % 2) + 2, :],
                                     start=(pc == 0), stop=(pc == 3),
                                     perf_mode=DR)
                for pc, est in ((0, es8), (1, es8), (2, esb), (3, esb)):
                    nc.tensor.matmul(pr[:], ones8[:],
                                     est[:, 2 * (pc % 2):2 * (pc % 2) + 2, :],
                                     start=(pc == 0), stop=(pc == 3),
                                     perf_mode=DR)
            else:
                for j in range(4):
                    nc.tensor.matmul(po[:], v_b[:, j, :], esb[:, j, :],
                                     start=(j == 0), stop=(j == 3))
                for j in range(4):
                    nc.tensor.matmul(pr[:], onesb[:, 0:1], esb[:, j, :],
                                     start=(j == 0), stop=(j == 3))
            r1r = atmp.tile([1, QB], F32, tag="r1r")
            nc.vector.reciprocal(r1r[:], pr[:])
            rb = atmp.tile([P, QB], F32, tag="rb")
            nc.gpsimd.partition_broadcast(rb[:], r1r[:])
            nc.vector.tensor_tensor(outT[:, h, qs], po[:], rb[:], mul)

        def oproj_tile(i, w_nts, psw, osb, blk):
            qt = blk * 2 + i // 4
            nt = i % 4
            ps = psw.tile([P, 512], F32, tag="psw")
            for h in range(H):
                nc.tensor.matmul(
                    ps[:], outT[:, h, qt * P:(qt + 1) * P],
                    w_nts[nt][:, h, :],
                    start=(h == 0), stop=(h == H - 1))
            ot = osb.tile([P, 512], F32, tag="ot")
            nc.vector.tensor_copy(ot[:], ps[:])
            nc.sync.dma_start(
                o_part[qt * P:(qt + 1) * P, nt * 512:(nt + 1) * 512],
                ot[:])

        with tc.tile_pool(name="expp", bufs=6) as expp, \
             tc.tile_pool(name="atmp", bufs=4) as atmp, \
             tc.tile_pool(name="ps_s", bufs=2, space="PSUM") as ps_s, \
             tc.tile_pool(name="ps_o", bufs=2, space="PSUM") as ps_o:
            w_nts = []
            for nt in range(4):
                w_nt = wop.tile([P, H, 512], BF, tag=f"wo{nt}",
                                name=f"wo_{nt}")
                nc.sync.dma_start(w_nt[:], wob[nt])
                w_nts.append(w_nt)
            # L phase: pure attention, software-pipelined depth 1
            prev = None
            for h in range(H):
                es = attn_scores(0, h, expp, ps_s)
                if prev is not None:
                    attn_po(0, prev[0], prev[1], atmp, ps_o)
                prev = (h, es)
            attn_po(0, prev[0], prev[1], atmp, ps_o)
            # H phase interleaved with L o_proj tiles, pipelined
            _MARK(nc, 'Hphase')
            with tc.tile_pool(name="ps_w", bufs=2, space="PSUM") as ps_w, \
                 tc.tile_pool(name="osb", bufs=3) as osb:
                prev = None
                for h in range(H):
                    es = attn_scores(1, h, expp, ps_s)
                    if prev is not None:
                        attn_po(1, prev[0], prev[1], atmp, ps_o)
                    prev = (h, es)
                    if h % 2 == 1:
                        oproj_tile(h // 2, w_nts, ps_w, osb, 0)
                attn_po(1, prev[0], prev[1], atmp, ps_o)
                for i in range(7):
                    oproj_tile(i, w_nts, ps_w, osb, 1)
                # last tile as two half-column tiles: the first half's
                # copy+DMA overlap the second half's matmuls, halving the
                # post-PE drain
                for sl in range(2):
                    cs = slice(sl * 256, (sl + 1) * 256)
                    ps = ps_w.tile([P, 256], F32, tag="psw")
                    for h in range(H):
                        nc.tensor.matmul(
                            ps[:], outT[:, h, 3 * P:4 * P],
                            w_nts[3][:, h, cs],
                            start=(h == 0), stop=(h == H - 1))
                    ot = osb.tile([P, 256], F32, tag=f"ot2_{sl}")
                    nc.vector.tensor_copy(ot[:], ps[:])
                    nc.sync.dma_start(
                        o_part[3 * P:4 * P,
                               3 * 512 + sl * 256:3 * 512 + (sl + 1) * 256],
                        ot[:])

    _MARK(nc, 'end')
    nc.finalize()
    return nc


def _prep_core_inputs(inputs):
    """Shard + lay out the full inputs for the 8 cores."""
    import ml_dtypes
    F8 = ml_dtypes.float8_e4m3
    F8E5 = ml_dtypes.float8_e5m2
    BF = ml_dtypes.bfloat16
    f32 = np.float32

    hs = np.asarray(inputs["hidden_states"], f32)
    w_qa = np.asarray(inputs["w_qa"], f32)
    w_qb = np.asarray(inputs["w_qb"], f32)
    w_kva = np.asarray(inputs["w_kva"], f32)
    w_kvb = np.asarray(inputs["w_kvb"], f32)
    qn_w = np.asarray(inputs["qn_w"], f32)
    kvn_w = np.asarray(inputs["kvn_w"], f32)
    w_o = np.asarray(inputs["w_o"], f32)
    att_mask = np.asarray(inputs["attention_mask"])
    for bname in ("b_qa", "b_qb", "b_kva", "b_kvb"):
        assert not np.asarray(inputs[bname], f32).any(), \
            "nonzero projection biases not supported"

    a = float(1.0 / (1.0 + np.exp(-f32(inputs["nope_logit"]))))
    g = float(1.0 / (1.0 + np.exp(-f32(inputs["rope_logit"]))))

    w_qb_f = qn_w[:, None] * w_qb
    w_kvb_f = kvn_w[:, None] * w_kvb

    wqa8 = np.ascontiguousarray(
        (w_qa * WS).reshape(DKD, 2, P, QRM, P).transpose(0, 2, 3, 1, 4)
    ).astype(F8)
    wqb8 = np.ascontiguousarray(
        (w_qb_f * WS).reshape(QRD, 2, P, H, P).transpose(3, 2, 0, 1, 4)
    ).astype(F8)
    wkva32 = w_kva * WS
    wkva_hi = wkva32.astype(F8)
    wkva_lo = (wkva32 - wkva_hi.astype(f32)).astype(F8)
    _wkl = lambda w: np.ascontiguousarray(
        w.reshape(DKD, 2, P, KVM, P).transpose(3, 2, 0, 1, 4)).astype(F8)
    wkv8h = _wkl(wkva_hi.astype(f32))
    wkv8l = _wkl(wkva_lo.astype(f32))
    wkvbb = np.ascontiguousarray(
        w_kvb_f.reshape(KVM, P, 2, P).transpose(1, 2, 0, 3)).astype(BF)
    wob = np.ascontiguousarray(
        w_o.reshape(H, P, 4, 512).transpose(2, 1, 0, 3)).astype(BF)
    gct = np.broadcast_to(
        np.array([WS * a, WS * (1.0 - a)], f32), (P, 2)).copy()

    cosb = g * np.asarray(inputs["cos_g"], f32) + (1 - g) * np.asarray(inputs["cos_l"], f32)
    sinb = g * np.asarray(inputs["sin_g"], f32) + (1 - g) * np.asarray(inputs["sin_l"], f32)

    i_p = np.arange(P)
    i_q = np.arange(QB)
    in_maps = []
    for c in range(NCORES):
        b, s = c // 2, c % 2
        blocks = [s, s + 2]
        qcols = np.concatenate([np.arange(bb * QB, (bb + 1) * QB)
                                for bb in blocks])
        hq8 = np.ascontiguousarray(
            hs[b][qcols, :].T.reshape(DKD, 2, P, TQ).transpose(2, 0, 1, 3)
        ).astype(F8)
        hkT = hs[b].T.reshape(DKD, 2, P, T).transpose(2, 0, 1, 3)
        hk_hi = np.ascontiguousarray(hkT).astype(F8)
        hk_lo = np.ascontiguousarray(
            hkT - hk_hi.astype(f32)).astype(F8)
        cb_q = cosb[b][qcols, :].T          # [64, TQ]
        sb_q = sinb[b][qcols, :].T
        cosq = np.ascontiguousarray(
            np.concatenate([cb_q, cb_q], 0)).astype(BF)
        sinq = np.ascontiguousarray(
            np.concatenate([sb_q, -sb_q], 0)).astype(BF)
        cb_k = cosb[b].T                    # [64, T]
        sb_k = sinb[b].T
        c1 = a + (1 - a) * cb_k
        s1 = (1 - a) * sb_k
        c1kv = np.ascontiguousarray(np.concatenate([c1, c1], 0))
        s1kv = np.ascontiguousarray(np.concatenate([s1, s1], 0))
        # masks: key > query  (+ padding): additive {0,-448} fp8, injected
        # through a 448*I DoubleRow stationary (-> -200704 in the psum,
        # exp arg -17.3, flushes to 0).  [P, 2(dbl-row), 4(chunk), QB],
        # second double-row slot zeroed.
        pad_b = (att_mask[b] == 0)
        use_pad = bool(pad_b.any())
        masks = []
        for mi, blk in enumerate(blocks):
            koff = mi * 512  # L-mask covers keys [0:512), H-mask [512:1024)
            key_abs = koff + (np.arange(4)[:, None, None] * P
                              + i_p[None, :, None])        # [4, P, 1]
            q_abs = blk * QB + i_q[None, None, :]          # [1, 1, QB]
            bad = (key_abs > q_abs) | pad_b[key_abs]
            m = (np.where(bad, -448.0, 0.0) + 0.0 * q_abs).transpose(1, 0, 2)
            m2 = np.zeros((P, 2, 4, QB), f32)
            m2[:, 0] = m
            masks.append(np.ascontiguousarray(m2).astype(F8E5))
        id2 = np.zeros((P, 2, P), f32)
        id2[:, 0, :] = 448.0 * np.eye(P)
        imap = {
            "hq8": hq8, "hk8h": hk_hi, "hk8l": hk_lo,
            "wqa8": wqa8, "wqb8": wqb8,
            "wkv8h": wkv8h, "wkv8l": wkv8l, "wkvbb": wkvbb, "wob": wob,
            "cosq": cosq, "sinq": sinq, "c1kv": c1kv, "s1kv": s1kv,
            "mask8l": masks[0], "mask8h": masks[1],
            "gct": gct,
            "id448": id2.astype(F8E5),
        }
        if use_pad:
            # pad-only mask for the H-block far pairs (keys 0-511)
            key_far = (np.arange(4)[:, None, None] * P
                       + i_p[None, :, None])
            mp = np.where(pad_b[key_far], NEG, 0.0) + np.zeros((1, 1, QB))
            imap["maskp"] = np.ascontiguousarray(
                mp.transpose(1, 0, 2)).astype(f32)
        in_maps.append(imap)
    return in_maps


NCORES = 8


def kernel(**inputs):
    use_pad = bool((np.asarray(inputs["attention_mask"]) == 0).any())
    if use_pad not in _nc_cache:
        _nc_cache[use_pad] = build_kernel(use_pad=use_pad)
    nc = _nc_cache[use_pad]

    from concourse.bass_utils import run_bass_kernel_spmd
    in_maps = _prep_core_inputs(inputs)
    res = run_bass_kernel_spmd(nc, in_maps, core_ids=list(range(NCORES)))
    out = np.empty((B, T, D), np.float32)
    for b in range(B):
        for s in range(2):
            r = res.results[2 * b + s]["o_part"]
            for i, blk in enumerate([s, s + 2]):
                out[b, blk * QB:(blk + 1) * QB] = r[i * QB:(i + 1) * QB]
    return out


# revision 34
# speedup vs baseline: 1.1418x; 1.0608x over previous
"""MLA/MQA attention (nn_Attention_33406255628587) on 8 TRN2 cores, v3.

Sharding: batch x4, query-halves x2.  Core (2b+s) handles batch b and
query blocks {s, s+2} (L=block s, H=block s+2) -- balanced causal load,
one uniform SPMD program; per-core causality lives in additive mask DATA
(key>query formula) applied to the L pairs and the last two H pairs.

Dtypes (validated numerically vs the 2e-2 gate):
  - q path fp8e4 DoubleRow (weights host-scaled x32; latents stored
    pre-normalized: rsbq multiplied in right after the down-proj)
  - kv chain bf16 (down/up, unnormalized latents; 1/rms folded into the
    kf rope tables per-column and into the v copy via per-partition scale)
  - scores always fp8 via u.w trick: scores = [q; q_pe].[a*kf; (1-a)*kf]
    (256-deep contraction -> DoubleRow)
  - attention po/pr: far pairs (H pairs 0-1) fp8 DoubleRow; own-block/near
    pairs (L pairs, H pairs 2-3) bf16 es+v  (early queries need precision)
  - o_proj bf16

v3 schedule (PE-idle driven rewrite of v2):
  qdown (dk-outer streams weights) -> q-up all 16 heads (wqb streamed
  right behind the wqa DMAs; rope/u8 conversion lags on DVE/Act into the
  next phase) -> kv-down in m-pair groups with hi/lo STREAM split (hi
  products first, lo-residual products after, so the lo DMAs can trail)
  -> kv-up: ss matmuls + rsqrt chain first, then k-up tn0 -> rope tn0 ->
  w8 tn0 (L attention + H far pairs only need keys 0-511, so L starts
  while the tn1 chain still runs), v-up + v copies between -> L
  attention (pure, sw-pipelined) -> H attention with L o_proj tiles
  interleaved -> H o_proj, last tile split in halves to shorten the
  drain.  kvnb psum->sbuf copies run on DVE (idle there), squares on
  Act, reciprocals emitted before the rope ops so the v/norm chain
  doesn't serialize behind them.
"""

import sys

sys.path.insert(0, "/opt/trn_rl_repo")

import numpy as np

B, T, D, H, HD = 4, 1024, 2048, 16, 128
QR, KVR = 1536, 512
EPS = 1e-6
NEG = -1000000000.0
SCALE = HD ** -0.5
WS = 32.0

P = 128
H2 = 64
TQ = 512          # queries per core
QB = 256          # query block
DKD = D // 256    # 8  dbl chunks of D
QRM = QR // P     # 12
QRD = QR // 256   # 6
KVM = KVR // P    # 4
DK = D // P       # 16
KT = T // P       # 8 key chunks

_nc_cache = {}

_MARKS = []


def _MARK(nc, label):
    _MARKS.append((label, nc.next_id()))


def build_kernel(use_pad=False):
    import concourse.bacc as bacc
    import concourse.tile as tile
    from concourse import mybir
    from contextlib import ExitStack

    F32 = mybir.dt.float32
    BF = mybir.dt.bfloat16
    F8 = mybir.dt.float8e4
    F8E5 = mybir.dt.float8e5
    AF = mybir.ActivationFunctionType
    DR = mybir.MatmulPerfMode.DoubleRow
    mul = mybir.AluOpType.mult
    add = mybir.AluOpType.add
    sub = mybir.AluOpType.subtract

    nc = bacc.Bacc("TRN2", target_bir_lowering=False, debug=False)

    # ---- DRAM I/O (host-prepared layouts, see _prep_core_inputs) ----
    hq8 = nc.dram_tensor("hq8", [P, DKD, 2, TQ], F8, kind="ExternalInput")
    hk8h = nc.dram_tensor("hk8h", [P, DKD, 2, T], F8, kind="ExternalInput")
    hk8l = nc.dram_tensor("hk8l", [P, DKD, 2, T], F8, kind="ExternalInput")
    wqa8 = nc.dram_tensor("wqa8", [DKD, P, QRM, 2, P], F8, kind="ExternalInput")
    wqb8 = nc.dram_tensor("wqb8", [H, P, QRD, 2, P], F8, kind="ExternalInput")
    wkv8h = nc.dram_tensor("wkv8h", [KVM, P, DKD, 2, P], F8,
                           kind="ExternalInput")
    wkv8l = nc.dram_tensor("wkv8l", [KVM, P, DKD, 2, P], F8,
                           kind="ExternalInput")
    wkvbb = nc.dram_tensor("wkvbb", [P, 2, KVM, P], BF, kind="ExternalInput")
    wob = nc.dram_tensor("wob", [4, P, H, 512], BF, kind="ExternalInput")
    cosq = nc.dram_tensor("cosq", [P, TQ], BF, kind="ExternalInput")
    sinq = nc.dram_tensor("sinq", [P, TQ], BF, kind="ExternalInput")
    c1kv = nc.dram_tensor("c1kv", [P, T], F32, kind="ExternalInput")
    s1kv = nc.dram_tensor("s1kv", [P, T], F32, kind="ExternalInput")
    mask8h = nc.dram_tensor("mask8h", [P, 2, 4, QB], F8E5,
                            kind="ExternalInput")
    gct = nc.dram_tensor("gct", [P, 2], F32, kind="ExternalInput")  # (32a, 32(1-a))
    if use_pad:
        maskp = nc.dram_tensor("maskp", [P, 4, QB], F32, kind="ExternalInput")
    id448 = nc.dram_tensor("id448", [P, 2, P], F8E5, kind="ExternalInput")
    mask8l = nc.dram_tensor("mask8l", [P, 2, 4, QB], F8E5,
                            kind="ExternalInput")
    o_part = nc.dram_tensor("o_part", [TQ, D], F32, kind="ExternalOutput")

    ESC = SCALE / (WS * WS)   # exp scale: scores psum = 1024 * true scores

    with tile.TileContext(nc, pool_alloc_mode="queue") as tc, ExitStack() as top:
        consts = top.enter_context(tc.tile_pool(name="consts", bufs=1))
        # persistent pools first (LIFO pool discipline: transient pools are
        # created after every long-lived one)
        latp = top.enter_context(tc.tile_pool(name="latp", bufs=1))
        kvnb = latp.tile([P, KVM, T], BF)      # 8KB/p unnormalized kv latents
        qn8 = latp.tile([P, QRD, 2, TQ], F8)   # 6KB/p NORMALIZED q latents
        normp = top.enter_context(tc.tile_pool(name="normp", bufs=1))
        rsbq = normp.tile([P, TQ], F32)        # 1/(32*rms_q), bcast partitions
        rsbkv = normp.tile([P, T], F32)        # 1/rms_kv, bcast partitions
        rsbkv_t = normp.tile([P, KT], F32)     # 1/rms_kv, keys on partitions
        wkvbp = top.enter_context(tc.tile_pool(name="wkvbp", bufs=1))
        wb = wkvbp.tile([P, 2, KVM, P], BF)
        kvp = top.enter_context(tc.tile_pool(name="kvp", bufs=1))
        w8 = kvp.tile([P, 2, T], F8)           # [a*kf; (1-a)*kf] x32
        v_b = kvp.tile([P, 4, P], BF)          # bf16 v, key chunks 0-3 (L po)
        v8h = kvp.tile([P, 4, 2, P], F8)       # fp8 v, all 8 chunks (H po)
        # q-up weights: pool reserved up-front (24KB/p) so their DMAs can
        # stream right behind the q-down weights
        wqbp = top.enter_context(tc.tile_pool(name="wqbp", bufs=1))
        qfp = top.enter_context(tc.tile_pool(name="qfp", bufs=1))
        u8 = qfp.tile([P, H, 2, TQ], F8)       # 16KB/p
        outp = top.enter_context(tc.tile_pool(name="outp", bufs=1))
        outT = outp.tile([P, H, TQ], BF)       # 16KB/p

        # ---- transient input pools ----
        dctx = ExitStack()
        wkvap = dctx.enter_context(tc.tile_pool(name="wkvap", bufs=1))
        hkp = dctx.enter_context(tc.tile_pool(name="hkp", bufs=1))
        hkh_sb = hkp.tile([P, DKD, 2, T], F8)  # 16KB/p
        hkl_sb = hkp.tile([P, DKD, 2, T], F8)  # 16KB/p
        # q-path pools last so they release (LIFO) right after q-down
        dctx_q = ExitStack()
        wqap = dctx_q.enter_context(tc.tile_pool(name="wqap", bufs=1))
        hqp = dctx_q.enter_context(tc.tile_pool(name="hqp", bufs=1))
        hq_sb = hqp.tile([P, DKD, 2, TQ], F8)  # 8KB/p

        # ---- input DMAs, one serial device, strict consumption order ----
        # qdown stream
        w_dk = []
        for dk in range(DKD):
            nc.sync.dma_start(hq_sb[:, dk, :, :], hq8[:, dk, :, :])
            w = wqap.tile([P, QRM, 2, P], F8, tag=f"wqa{dk}", name=f"wqa_{dk}")
            nc.sync.dma_start(w[:], wqa8[dk])
            w_dk.append(w)
        # q-up needs cos/sin + wqb right after qdown: stream them next
        cq_t = consts.tile([P, TQ], BF)        # raw blended cos/sin for q
        sq_t = consts.tile([P, TQ], BF)
        nc.sync.dma_start(cq_t[:], cosq[:])
        nc.sync.dma_start(sq_t[:], sinq[:])
        w_hs = []
        for h in range(H):
            w_h = wqbp.tile([P, QRD, 2, P], F8, tag=f"wqb{h}",
                            name=f"wqb_{h}")
            nc.sync.dma_start(w_h[:], wqb8[h])
            w_hs.append(w_h)
        gc_sb = consts.tile([P, 2], F32)
        nc.scalar.dma_start(gc_sb[:], gct[:])
        id_sb = consts.tile([P, 2, P], F8E5)   # 448*I in row 0, zeros row 1
        nc.scalar.dma_start(id_sb[:], id448[:])
        # kv-down stream, matching the m-pair hi->lo consumption order:
        # wvh01, hkh, hkl, wvl01, wvh23, wvl23
        w_kvh, w_kvl = [], []
        for m_ in range(2):
            wvh = wkvap.tile([P, DKD, 2, P], F8, tag=f"wkvh{m_}",
                             name=f"wkvh_{m_}")
            nc.sync.dma_start(wvh[:], wkv8h[m_])
            w_kvh.append(wvh)
        for dk in range(DKD):
            nc.sync.dma_start(hkh_sb[:, dk, :, :], hk8h[:, dk, :, :])
        for dk in range(DKD):
            nc.sync.dma_start(hkl_sb[:, dk, :, :], hk8l[:, dk, :, :])
        for m_ in range(2):
            wvl = wkvap.tile([P, DKD, 2, P], F8, tag=f"wkvl{m_}",
                             name=f"wkvl_{m_}")
            nc.sync.dma_start(wvl[:], wkv8l[m_])
            w_kvl.append(wvl)
        for m_ in range(2, KVM):
            wvh = wkvap.tile([P, DKD, 2, P], F8, tag=f"wkvh{m_}",
                             name=f"wkvh_{m_}")
            nc.sync.dma_start(wvh[:], wkv8h[m_])
            w_kvh.append(wvh)
        for m_ in range(2, KVM):
            wvl = wkvap.tile([P, DKD, 2, P], F8, tag=f"wkvl{m_}",
                             name=f"wkvl_{m_}")
            nc.sync.dma_start(wvl[:], wkv8l[m_])
            w_kvl.append(wvl)
        nc.sync.dma_start(wb[:], wkvbb[:])
        ck_raw = consts.tile([P, T], F32)
        sk_raw = consts.tile([P, T], F32)
        nc.sync.dma_start(ck_raw[:], c1kv[:])
        nc.sync.dma_start(sk_raw[:], s1kv[:])
        ml_sb = consts.tile([P, 2, 4, QB], F8E5)
        mh_sb = consts.tile([P, 2, 4, QB], F8E5)
        nc.sync.dma_start(ml_sb[:], mask8l[:])
        nc.sync.dma_start(mh_sb[:], mask8h[:])
        if use_pad:
            mp_sb = consts.tile([P, 4, QB], F32)
            nc.sync.dma_start(mp_sb[:], maskp[:])
        ones8w = consts.tile([P, 2, P], F8)
        nc.vector.memset(ones8w[:], 1.0)
        ones8 = ones8w[:, :, 0:1]
        onesb = consts.tile([P, 2], BF)
        nc.vector.memset(onesb[:], 1.0)
        eps_sb = consts.tile([P, 1], F32)
        nc.vector.memset(eps_sb[:], 1024.0 * EPS)
        epsn_sb = consts.tile([P, 1], F32)
        nc.vector.memset(epsn_sb[:], EPS)

        _MARK(nc, 'qdown')
        # ---------- q down-proj (fp8 DoubleRow, dk-outer streaming) ----------
        # Two halves of 6 m-chunks so the 6 live psums (+ss) fit in 8 banks;
        # dk-outer order lets the PE consume weight chunks as they stream in.
        with tc.tile_pool(name="sqq", bufs=1) as sqqp, \
             tc.tile_pool(name="qltmp", bufs=1) as qltmp, \
             tc.tile_pool(name="ps_qd", bufs=1, space="PSUM") as psqd, \
             tc.tile_pool(name="ps_ssq", bufs=1, space="PSUM") as psssq:
            ss_q = psssq.tile([P, TQ], F32)
            sq_m = [sqqp.tile([P, 2, TQ], F8, tag=f"sqq{dm}",
                              name=f"sqq_{dm}") for dm in range(QRD)]
            # q_lat parked in SBUF bf16 so psums free up per half (the
            # latents end up fp8 in qn8, so bf16 parking costs nothing)
            qlat = qltmp.tile([P, QRM, TQ], BF)    # 12KB/p, freed after qn8
            for half in range(2):
                ms = list(range(6 * half, 6 * half + 6))
                ps_m = {m: psqd.tile([P, TQ], F32, tag=f"psqd{m % 6}",
                                     name=f"psqd_{m}") for m in ms}
                for dk in range(DKD):
                    for m in ms:
                        for tq in range(2):
                            ts = slice(tq * 256, (tq + 1) * 256)
                            nc.tensor.matmul(
                                ps_m[m][:, ts], w_dk[dk][:, m, :, :],
                                hq_sb[:, dk, :, ts],
                                start=(dk == 0 and tq == 0),
                                stop=(dk == DKD - 1 and tq == 1),
                                perf_mode=DR)
                for m in ms:
                    nc.scalar.activation(sq_m[m // 2][:, m % 2, :],
                                         ps_m[m][:], AF.Square,
                                         scale=1.0 / WS)
                    nc.vector.tensor_copy(qlat[:, m, :], ps_m[m][:])
                # this half's ss_q contribution (accumulated across halves)
                for dm in range(3 * half, 3 * half + 3):
                    for tq in range(2):
                        ts = slice(tq * 256, (tq + 1) * 256)
                        nc.tensor.matmul(
                            ss_q[:, ts], ones8w[:], sq_m[dm][:, :, ts],
                            start=(half == 0 and dm == 0 and tq == 0),
                            stop=(half == 1 and dm == 5 and tq == 1),
                            perf_mode=DR)
            nc.scalar.activation(rsbq[:], ss_q[:], AF.Sqrt,
                                 bias=eps_sb[:], scale=1024.0 / QR)
            nc.vector.reciprocal(rsbq[:], rsbq[:])
            # qn8 = q_lat * rsbq  (pre-normalized latents); DVE/Pool split
            # so the serial tail gating q-up stays short
            for m in range(QRM):
                eng = nc.vector if m < 8 else nc.gpsimd
                eng.tensor_tensor(qn8[:, m // 2, m % 2, :],
                                  qlat[:, m, :], rsbq[:], mul)
        dctx_q.close()

        # ---------- q up-proj (interleaved into kv-down PE stream) ----------
        # Rope per head: Act makes a bf16 psum copy (psb) + the final fp8
        # cast; DVE does the fp8 u0 copy, cos-product, one sin half and the
        # combine (all-bf16 SBUF -> 2x mode); Pool (gpsimd, otherwise idle)
        # takes the other sin half.  The psum frees after the two copies,
        # so the q-up pipeline never waits on the rope tail.
        _MARK(nc, 'qup')

        def q_up(h, rtmpq, psqu):
            ps = psqu.tile([P, TQ], F32, tag="psqu")
            for tq in range(2):
                ts = slice(tq * 256, (tq + 1) * 256)
                for dm in range(QRD):
                    nc.tensor.matmul(ps[:, ts], w_hs[h][:, dm, :, :],
                                     qn8[:, dm, :, ts],
                                     start=(dm == 0), stop=(dm == QRD - 1),
                                     perf_mode=DR)
            nc.scalar.copy(u8[:, h, 0, :], ps[:])
            ta = rtmpq.tile([P, TQ], BF, tag="ta")
            tb = rtmpq.tile([P, TQ], BF, tag="tb")
            # rope products on DVE straight from the psum (partition-shifted
            # reads are only legal from PSUM); the aligned combine runs on
            # Pool (gpsimd: SBUF-only, aligned-only) and writes the fp8
            # u8 slot directly
            nc.vector.tensor_tensor(ta[:], ps[:], cq_t[:], mul)
            nc.vector.tensor_tensor(tb[0:H2, :], ps[H2:P, :],
                                    sq_t[0:H2, :], mul)
            nc.vector.tensor_tensor(tb[H2:P, :], ps[0:H2, :],
                                    sq_t[H2:P, :], mul)
            nc.gpsimd.tensor_tensor(u8[:, h, 1, :], ta[:], tb[:], sub)

        # ---------- kv down-proj: m-pair groups, hi stream then lo ----------
        with tc.tile_pool(name="sqkv", bufs=2) as sqkvp:
          with tc.tile_pool(name="rtmpq", bufs=4) as rtmpq, \
               tc.tile_pool(name="ps_qu", bufs=3, space="PSUM") as psqu:
            qup_h = iter(range(H))
            _MARK(nc, 'kvdown')
            with tc.tile_pool(name="ps_kvd", bufs=1, space="PSUM") as pskvd:
                sq_kv = [sqkvp.tile([P, 2, T], F8, tag=f"sqkv{dm}",
                                    name=f"sqkv_{dm}") for dm in range(2)]
                unit = 0
                for mm in ((0, 1), (2, 3)):
                    mp = 0 if mm == (0, 1) else 1
                    ps_pair = {m: pskvd.tile([P, T], F32, tag=f"pskvd{m % 2}",
                                             name=f"pskvd_{m}") for m in mm}
                    # hi*hi stream (dk-outer so the PE tracks the hkh DMAs),
                    # then the two lo-residual streams (their DMAs trail);
                    # one q-up head woven in every 3 dk-units
                    for si, (hs_, wgetter, st0, st1) in enumerate((
                            (hkh_sb, lambda m: w_kvh[m], True, False),
                            (hkl_sb, lambda m: w_kvh[m], False, False),
                            (hkh_sb, lambda m: w_kvl[m], False, True))):
                        for dk in range(DKD):
                            for m in mm:
                                for sl in range(4):
                                    ts = slice(sl * 256, (sl + 1) * 256)
                                    # psum zero regions are 2KB banks: one
                                    # start/stop per 512-col bank
                                    nc.tensor.matmul(
                                        ps_pair[m][:, ts],
                                        wgetter(m)[:, dk, :, :],
                                        hs_[:, dk, :, ts],
                                        start=(st0 and dk == 0
                                               and sl % 2 == 0),
                                        stop=(st1 and dk == DKD - 1
                                              and sl % 2 == 1),
                                        perf_mode=DR)
                            # pair0-hi runs clean (it's DMA-paced and the
                            # q-up heads would stall on the qn8 tail ahead
                            # of it in the in-order PE queue); heads weave
                            # into the later streams every 2 dk-units
                            if mp == 0 and si == 0:
                                continue
                            unit += 1
                            if unit % 2 == 0:
                                h = next(qup_h, None)
                                if h is not None:
                                    q_up(h, rtmpq, psqu)
                    for m in mm:
                        # latent tn0 halves first: they gate the long chain
                        # k-tn0 -> rope -> w8 -> L attention
                        nc.scalar.copy(kvnb[:, m, 0:512],
                                       ps_pair[m][:, 0:512])
                    for m in mm:
                        nc.scalar.activation(sq_kv[m // 2][:, m % 2, :],
                                             ps_pair[m][:], AF.Square,
                                             scale=1.0 / WS)
                    if mm == (2, 3):
                        # last pair: tn1 halves split Act/DVE so the Act
                        # queue reaches the kvup sqrt sooner
                        nc.scalar.copy(kvnb[:, 2, 512:T],
                                       ps_pair[2][:, 512:T])
                        nc.vector.tensor_copy(kvnb[:, 3, 512:T],
                                              ps_pair[3][:, 512:T])
                    else:
                        for m in mm:
                            nc.scalar.copy(kvnb[:, m, 512:T],
                                           ps_pair[m][:, 512:T])
                for h in qup_h:
                    q_up(h, rtmpq, psqu)

          # ---------- kv up-proj + norm chain ----------
          if True:
            _MARK(nc, 'kvup')
            with tc.tile_pool(name="kfp", bufs=1) as kfp, \
                 tc.tile_pool(name="rtmp", bufs=2) as rtmp, \
                 tc.tile_pool(name="ps_sskv", bufs=1, space="PSUM") as pssskv, \
                 tc.tile_pool(name="ps_sst", bufs=1, space="PSUM") as psst, \
                 tc.tile_pool(name="ps_k", bufs=1, space="PSUM") as psk, \
                 tc.tile_pool(name="ps_v", bufs=1, space="PSUM") as psv:
                ss_kv = pssskv.tile([P, 2, 512], F32)
                ss_t = psst.tile([P, KT], F32)
                kf = kfp.tile([P, T], F32)
                ps_vt = psv.tile([P, KT, P], F32)   # all 8 v chunks, 2 banks
                for dm in range(2):
                    for tq in range(4):
                        ts = slice(tq * 256, (tq + 1) * 256)
                        tnn, to = tq // 2, (tq % 2) * 256
                        nc.tensor.matmul(
                            ss_kv[:, tnn, to:to + 256], ones8w[:],
                            sq_kv[dm][:, :, ts],
                            start=(dm == 0 and to == 0),
                            stop=(dm == 1 and to == 256), perf_mode=DR)
                    for kc in range(KT):
                        ks = slice(kc * P, (kc + 1) * P)
                        nc.tensor.matmul(
                            ss_t[:, kc:kc + 1], sq_kv[dm][:, :, ks],
                            ones8[:],
                            start=(dm == 0 and kc == 0),
                            stop=(dm == 1 and kc == KT - 1), perf_mode=DR)
                for tn in range(2):
                    nc.scalar.activation(rsbkv[:, tn * 512:(tn + 1) * 512],
                                         ss_kv[:, tn, :], AF.Sqrt,
                                         bias=eps_sb[:], scale=1024.0 / KVR)
                nc.scalar.activation(rsbkv_t[:], ss_t[:], AF.Sqrt,
                                     bias=eps_sb[:], scale=1024.0 / KVR)
                recips_done = [False]
                for tn in range(2):
                    ts = slice(tn * 512, (tn + 1) * 512)
                    ps = psk.tile([P, 512], F32, tag="psk")
                    for m in range(KVM):
                        nc.tensor.matmul(ps[:], wb[:, 0, m, :],
                                         kvnb[:, m, ts],
                                         start=(m == 0), stop=(m == KVM - 1))
                    # kf = rope_gate(ps); raw tables (rsbkv folded below)
                    ta = rtmp.tile([P, 512], F32, tag="ta")
                    tb = rtmp.tile([P, 512], F32, tag="tb")
                    nc.vector.tensor_tensor(ta[:], ps[:], ck_raw[:, ts], mul)
                    nc.vector.tensor_tensor(tb[0:H2, :], ps[H2:P, :],
                                            sk_raw[0:H2, ts], mul)
                    nc.vector.tensor_tensor(tb[H2:P, :], ps[0:H2, :],
                                            sk_raw[H2:P, ts], mul)
                    nc.vector.tensor_tensor(kf[0:H2, ts], ta[0:H2, :],
                                            tb[0:H2, :], sub)
                    nc.vector.tensor_tensor(kf[H2:P, ts], ta[H2:P, :],
                                            tb[H2:P, :], add)
                    if not recips_done[0]:
                        # recips emitted after the first rope products so
                        # the DVE works while the Act sqrt completes
                        recips_done[0] = True
                        nc.vector.reciprocal(rsbkv[:, 0:512],
                                             rsbkv[:, 0:512])
                        nc.vector.reciprocal(rsbkv_t[:], rsbkv_t[:])
                        nc.vector.reciprocal(rsbkv[:, 512:T],
                                             rsbkv[:, 512:T])
                    # fold rsbkv + write both w8 gate rows for this tn so
                    # the L phase (keys 0-511) unblocks after tn=0
                    nc.vector.tensor_tensor(kf[:, ts], kf[:, ts],
                                            rsbkv[:, ts], mul)
                    nc.vector.tensor_scalar(out=w8[:, 0, ts], in0=kf[:, ts],
                                            scalar1=gc_sb[:, 0:1],
                                            scalar2=None, op0=mul)
                    nc.vector.tensor_scalar(out=w8[:, 1, ts], in0=kf[:, ts],
                                            scalar1=gc_sb[:, 1:2],
                                            scalar2=None, op0=mul)
                    # v up-proj for this tn's key chunks right after the
                    # k chain; Act does the per-partition rms scale
                    for kc in range(4 * tn, 4 * tn + 4):
                        ks = slice(kc * P, (kc + 1) * P)
                        for m in range(KVM):
                            nc.tensor.matmul(ps_vt[:, kc, :],
                                             kvnb[:, m, ks],
                                             wb[:, 1, m, :],
                                             start=(m == 0),
                                             stop=(m == KVM - 1))
                    for kc in range(4 * tn, 4 * tn + 4):
                        nc.scalar.activation(v8h[:, kc // 2, kc % 2, :],
                                             ps_vt[:, kc, :],
                                             AF.Identity,
                                             scale=rsbkv_t[:, kc:kc + 1])
                        if tn == 0:
                            nc.scalar.activation(v_b[:, kc, :],
                                                 ps_vt[:, kc, :],
                                                 AF.Identity,
                                                 scale=rsbkv_t[:, kc:kc + 1])
        dctx.close()

        # ---------- attention + o_proj (sw-pipelined) ----------
        _MARK(nc, 'Lphase')
        wop = top.enter_context(tc.tile_pool(name="wop", bufs=1))

        def attn_scores(blk, h, expp, ps_s):
            """Emit scores+mask+exp for head h; returns es tiles.

            Masks are additive on the PE: a DoubleRow inject of the fp8
            {0,-448} mask through the 448*I stationary adds -200704 to the
            psum (exp arg -17.3 -> flushes to exactly 0 in fp8/bf16).
            L keeps bf16 es (early queries: few-key softmax needs the
            precision); H is all-fp8 so po/pr run DoubleRow.
            """
            qs = slice(blk * QB, (blk + 1) * QB)
            es8 = None
            if blk:  # far pairs 0-1, fp8, no causal mask needed
                pss = ps_s.tile([P, 4, QB], F32, tag="pss")
                for kc in range(4):
                    nc.tensor.matmul(pss[:, kc, :],
                                     w8[:, :, kc * P:(kc + 1) * P],
                                     u8[:, h, :, qs], start=True,
                                     stop=True, perf_mode=DR)
                if use_pad:
                    nc.vector.tensor_tensor(pss[:], pss[:], mp_sb[:], add)
                es8 = expp.tile([P, 4, QB], F8, tag="es8")
                nc.scalar.activation(es8[:], pss[:], AF.Exp, bias=0.0,
                                     scale=ESC)
            k0 = 4 if blk else 0
            msk = mh_sb if blk else ml_sb
            pss = ps_s.tile([P, 4, QB], F32, tag="pss")
            for j in range(4):
                kc = k0 + j
                nc.tensor.matmul(pss[:, j, :],
                                 w8[:, :, kc * P:(kc + 1) * P],
                                 u8[:, h, :, qs], start=True,
                                 stop=False, perf_mode=DR)
                nc.tensor.matmul(pss[:, j, :], id_sb[:],
                                 msk[:, :, j, :], start=False, stop=True,
                                 perf_mode=DR)
            esb = expp.tile([P, 4, QB], BF if blk == 0 else F8, tag="esb")
            nc.scalar.activation(esb[:], pss[:], AF.Exp, bias=0.0,
                                 scale=ESC)
            return es8, esb

        def attn_po(blk, h, es, atmp, ps_o):
            """po group then pr group (sequential groups, shared region).
            H (blk=1): everything fp8 DoubleRow (es8 far + fp8 near + v8h).
            L (blk=0): bf16 es x bf16 v (early-query precision)."""
            qs = slice(blk * QB, (blk + 1) * QB)
            es8, esb = es
            po_t = ps_o.tile([P, 2, QB], F32, tag="po")
            po = po_t[:, 0, :]
            pr = po_t[0:1, 1, :]
            if blk:
                for pc, est in ((0, es8), (1, es8), (2, esb), (3, esb)):
                    nc.tensor.matmul(po[:], v8h[:, pc, :, :],
                                     est[:, 2 * (pc % 2):2 * (pc % 2) + 2, :],
                                     start=(pc == 0), stop=(pc == 3),
                                     perf_mode=DR)
                for pc, est in ((0, es8), (1, es8), (2, esb), (3, esb)):
                    nc.tensor.matmul(pr[:], ones8[:],
                                     est[:, 2 * (pc % 2):2 * (pc % 2) + 2, :],
                                     start=(pc == 0), stop=(pc == 3),
                                     perf_mode=DR)
            else:
                for j in range(4):
                    nc.tensor.matmul(po[:], v_b[:, j, :], esb[:, j, :],
                                     start=(j == 0), stop=(j == 3))
                for j in range(4):
                    nc.tensor.matmul(pr[:], onesb[:, 0:1], esb[:, j, :],
                                     start=(j == 0), stop=(j == 3))
            r1r = atmp.tile([1, QB], F32, tag="r1r")
            nc.vector.reciprocal(r1r[:], pr[:])
            rb = atmp.tile([P, QB], F32, tag="rb")
            nc.gpsimd.partition_broadcast(rb[:], r1r[:])
            nc.vector.tensor_tensor(outT[:, h, qs], po[:], rb[:], mul)

        def oproj_tile(i, w_nts, psw, osb, blk):
            qt = blk * 2 + i // 4
            nt = i % 4
            ps = psw.tile([P, 512], F32, tag="psw")
            for h in range(H):
                nc.tensor.matmul(
                    ps[:], outT[:, h, qt * P:(qt + 1) * P],
                    w_nts[nt][:, h, :],
                    start=(h == 0), stop=(h == H - 1))
            ot = osb.tile([P, 512], F32, tag="ot")
            nc.vector.tensor_copy(ot[:], ps[:])
            nc.sync.dma_start(
                o_part[qt * P:(qt + 1) * P, nt * 512:(nt + 1) * 512],
                ot[:])

        with tc.tile_pool(name="expp", bufs=6) as expp, \
             tc.tile_pool(name="atmp", bufs=4) as atmp, \
             tc.tile_pool(name="ps_s", bufs=2, space="PSUM") as ps_s, \
             tc.tile_pool(name="ps_o", bufs=2, space="PSUM") as ps_o:
            w_nts = []
            for nt in range(4):
                w_nt = wop.tile([P, H, 512], BF, tag=f"wo{nt}",
                                name=f"wo_{nt}")
                nc.sync.dma_start(w_nt[:], wob[nt])
                w_nts.append(w_nt)
            # L phase: pure attention, software-pipelined depth 1
            prev = None
            for h in range(H):
                es = attn_scores(0, h, expp, ps_s)
                if prev is not None:
                    attn_po(0, prev[0], prev[1], atmp, ps_o)
                prev = (h, es)
            attn_po(0, prev[0], prev[1], atmp, ps_o)
            # H phase interleaved with L o_proj tiles, pipelined
            _MARK(nc, 'Hphase')
            with tc.tile_pool(name="ps_w", bufs=2, space="PSUM") as ps_w, \
                 tc.tile_pool(name="osb", bufs=3) as osb:
                prev = None
                for h in range(H):
                    es = attn_scores(1, h, expp, ps_s)
                    if prev is not None:
                        attn_po(1, prev[0], prev[1], atmp, ps_o)
                    prev = (h, es)
                    if h % 2 == 1:
                        oproj_tile(h // 2, w_nts, ps_w, osb, 0)
                attn_po(1, prev[0], prev[1], atmp, ps_o)
                for i in range(7):
                    oproj_tile(i, w_nts, ps_w, osb, 1)
                # last tile as two half-column tiles: the first half's
                # copy+DMA overlap the second half's matmuls, halving the
                # post-PE drain
                for sl in range(2):
                    cs = slice(sl * 256, (sl + 1) * 256)
                    ps = ps_w.tile([P, 256], F32, tag="psw")
                    for h in range(H):
                        nc.tensor.matmul(
                            ps[:], outT[:, h, 3 * P:4 * P],
                            w_nts[3][:, h, cs],
                            start=(h == 0), stop=(h == H - 1))
                    ot = osb.tile([P, 256], F32, tag=f"ot2_{sl}")
                    nc.vector.tensor_copy(ot[:], ps[:])
                    nc.sync.dma_start(
                        o_part[3 * P:4 * P,
                               3 * 512 + sl * 256:3 * 512 + (sl + 1) * 256],
                        ot[:])

    _MARK(nc, 'end')
    nc.finalize()
    return nc


def _prep_core_inputs(inputs):
    """Shard + lay out the full inputs for the 8 cores."""
    import ml_dtypes
    F8 = ml_dtypes.float8_e4m3
    F8E5 = ml_dtypes.float8_e5m2
    BF = ml_dtypes.bfloat16
    f32 = np.float32

    hs = np.asarray(inputs["hidden_states"], f32)
    w_qa = np.asarray(inputs["w_qa"], f32)
    w_qb = np.asarray(inputs["w_qb"], f32)
    w_kva = np.asarray(inputs["w_kva"], f32)
    w_kvb = np.asarray(inputs["w_kvb"], f32)
    qn_w = np.asarray(inputs["qn_w"], f32)
    kvn_w = np.asarray(inputs["kvn_w"], f32)
    w_o = np.asarray(inputs["w_o"], f32)
    att_mask = np.asarray(inputs["attention_mask"])
    for bname in ("b_qa", "b_qb", "b_kva", "b_kvb"):
        assert not np.asarray(inputs[bname], f32).any(), \
            "nonzero projection biases not supported"

    a = float(1.0 / (1.0 + np.exp(-f32(inputs["nope_logit"]))))
    g = float(1.0 / (1.0 + np.exp(-f32(inputs["rope_logit"]))))

    w_qb_f = qn_w[:, None] * w_qb
    w_kvb_f = kvn_w[:, None] * w_kvb

    wqa8 = np.ascontiguousarray(
        (w_qa * WS).reshape(DKD, 2, P, QRM, P).transpose(0, 2, 3, 1, 4)
    ).astype(F8)
    wqb8 = np.ascontiguousarray(
        (w_qb_f * WS).reshape(QRD, 2, P, H, P).transpose(3, 2, 0, 1, 4)
    ).astype(F8)
    wkva32 = w_kva * WS
    wkva_hi = wkva32.astype(F8)
    wkva_lo = (wkva32 - wkva_hi.astype(f32)).astype(F8)
    _wkl = lambda w: np.ascontiguousarray(
        w.reshape(DKD, 2, P, KVM, P).transpose(3, 2, 0, 1, 4)).astype(F8)
    wkv8h = _wkl(wkva_hi.astype(f32))
    wkv8l = _wkl(wkva_lo.astype(f32))
    wkvbb = np.ascontiguousarray(
        w_kvb_f.reshape(KVM, P, 2, P).transpose(1, 2, 0, 3)).astype(BF)
    wob = np.ascontiguousarray(
        w_o.reshape(H, P, 4, 512).transpose(2, 1, 0, 3)).astype(BF)
    gct = np.broadcast_to(
        np.array([WS * a, WS * (1.0 - a)], f32), (P, 2)).copy()

    cosb = g * np.asarray(inputs["cos_g"], f32) + (1 - g) * np.asarray(inputs["cos_l"], f32)
    sinb = g * np.asarray(inputs["sin_g"], f32) + (1 - g) * np.asarray(inputs["sin_l"], f32)

    i_p = np.arange(P)
    i_q = np.arange(QB)
    in_maps = []
    for c in range(NCORES):
        b, s = c // 2, c % 2
        blocks = [s, s + 2]
        qcols = np.concatenate([np.arange(bb * QB, (bb + 1) * QB)
                                for bb in blocks])
        hq8 = np.ascontiguousarray(
            hs[b][qcols, :].T.reshape(DKD, 2, P, TQ).transpose(2, 0, 1, 3)
        ).astype(F8)
        hkT = hs[b].T.reshape(DKD, 2, P, T).transpose(2, 0, 1, 3)
        hk_hi = np.ascontiguousarray(hkT).astype(F8)
        hk_lo = np.ascontiguousarray(
            hkT - hk_hi.astype(f32)).astype(F8)
        cb_q = cosb[b][qcols, :].T          # [64, TQ]
        sb_q = sinb[b][qcols, :].T
        cosq = np.ascontiguousarray(
            np.concatenate([cb_q, cb_q], 0)).astype(BF)
        sinq = np.ascontiguousarray(
            np.concatenate([sb_q, -sb_q], 0)).astype(BF)
        cb_k = cosb[b].T                    # [64, T]
        sb_k = sinb[b].T
        c1 = a + (1 - a) * cb_k
        s1 = (1 - a) * sb_k
        c1kv = np.ascontiguousarray(np.concatenate([c1, c1], 0))
        s1kv = np.ascontiguousarray(np.concatenate([s1, s1], 0))
        # masks: key > query  (+ padding): additive {0,-448} fp8, injected
        # through a 448*I DoubleRow stationary (-> -200704 in the psum,
        # exp arg -17.3, flushes to 0).  [P, 2(dbl-row), 4(chunk), QB],
        # second double-row slot zeroed.
        pad_b = (att_mask[b] == 0)
        use_pad = bool(pad_b.any())
        masks = []
        for mi, blk in enumerate(blocks):
            koff = mi * 512  # L-mask covers keys [0:512), H-mask [512:1024)
            key_abs = koff + (np.arange(4)[:, None, None] * P
                              + i_p[None, :, None])        # [4, P, 1]
            q_abs = blk * QB + i_q[None, None, :]          # [1, 1, QB]
            bad = (key_abs > q_abs) | pad_b[key_abs]
            m = (np.where(bad, -448.0, 0.0) + 0.0 * q_abs).transpose(1, 0, 2)
            m2 = np.zeros((P, 2, 4, QB), f32)
            m2[:, 0] = m
            masks.append(np.ascontiguousarray(m2).astype(F8E5))
        id2 = np.zeros((P, 2, P), f32)
        id2[:, 0, :] = 448.0 * np.eye(P)
        imap = {
            "hq8": hq8, "hk8h": hk_hi, "hk8l": hk_lo,
            "wqa8": wqa8, "wqb8": wqb8,
            "wkv8h": wkv8h, "wkv8l": wkv8l, "wkvbb": wkvbb, "wob": wob,
            "cosq": cosq, "sinq": sinq, "c1kv": c1kv, "s1kv": s1kv,
            "mask8l": masks[0], "mask8h": masks[1],
            "gct": gct,
            "id448": id2.astype(F8E5),
        }
        if use_pad:
            # pad-only mask for the H-block far pairs (keys 0-511)
            key_far = (np.arange(4)[:, None, None] * P
                       + i_p[None, :, None])
            mp = np.where(pad_b[key_far], NEG, 0.0) + np.zeros((1, 1, QB))
            imap["maskp"] = np.ascontiguousarray(
                mp.transpose(1, 0, 2)).astype(f32)
        in_maps.append(imap)
    return in_maps


NCORES = 8


def kernel(**inputs):
    use_pad = bool((np.asarray(inputs["attention_mask"]) == 0).any())
    if use_pad not in _nc_cache:
        _nc_cache[use_pad] = build_kernel(use_pad=use_pad)
    nc = _nc_cache[use_pad]

    from concourse.bass_utils import run_bass_kernel_spmd
    in_maps = _prep_core_inputs(inputs)
    res = run_bass_kernel_spmd(nc, in_maps, core_ids=list(range(NCORES)))
    out = np.empty((B, T, D), np.float32)
    for b in range(B):
        for s in range(2):
            r = res.results[2 * b + s]["o_part"]
            for i, blk in enumerate([s, s + 2]):
                out[b, blk * QB:(blk + 1) * QB] = r[i * QB:(i + 1) * QB]
    return out


# revision 39
# speedup vs baseline: 1.1505x; 1.0076x over previous
"""MLA/MQA attention (nn_Attention_33406255628587) on 8 TRN2 cores, v3.

Sharding: batch x4, query-halves x2.  Core (2b+s) handles batch b and
query blocks {s, s+2} (L=block s, H=block s+2) -- balanced causal load,
one uniform SPMD program; per-core causality lives in additive mask DATA
(key>query formula).

Dtypes (validated on hw: rel err ~7.0e-3 vs the 2e-2 gate):
  - q path fp8e4 DoubleRow (weights host-scaled x32; latents stored
    pre-normalized); kv down-proj fp8 hi/lo residual streams
  - scores fp8 via u.w trick: scores = [q; q_pe].[a*kf; (1-a)*kf]
    (256-deep contraction -> DoubleRow)
  - causal/pad masks: additive {0,-448} fp8e5m2 chunks injected on the
    PE through a 448*I DoubleRow stationary (-200704 in the psum ->
    exp arg -17.3 -> flushes to exact 0); no DVE masking
  - L attention (early queries, few-key softmax): bf16 es x bf16 v
  - H attention: all-fp8 DoubleRow (es8 + v8h) for scores/po/pr
  - o_proj bf16

Schedule (PE-idle driven):
  qdown (dk-outer streams weights; ss_q accumulated per half; qn8 tail
  split DVE/Pool) -> kv-down in m-pair groups with hi/lo STREAM split
  (hi products first, lo residuals after, so the lo DMAs can trail),
  with all 16 q-up heads woven in every 2 dk-units after the pair-0 hi
  stream; q-up rope: Act psum->fp8 copy, DVE psum-sourced shifted
  products (SBUF shifts are illegal on trn2), Pool (gpsimd) fuses the
  combine + fp8 cast straight into u8 -> kv-up: ss -> sqrt -> recips
  (emitted mid-rope), k-up tn0 -> rope tn0 -> w8 tn0 first so the L
  phase (keys 0-511) starts while tn1 still runs; exp act-table
  pre-pulled -> L attention (pure, sw-pipelined, carried across the
  L/H boundary) -> H attention with L o_proj tiles interleaved -> H
  o_proj, last tile split in halves to shorten the drain.
"""

import sys

sys.path.insert(0, "/opt/trn_rl_repo")

import numpy as np

B, T, D, H, HD = 4, 1024, 2048, 16, 128
QR, KVR = 1536, 512
EPS = 1e-6
NEG = -1000000000.0
SCALE = HD ** -0.5
WS = 32.0

P = 128
H2 = 64
TQ = 512          # queries per core
QB = 256          # query block
DKD = D // 256    # 8  dbl chunks of D
QRM = QR // P     # 12
QRD = QR // 256   # 6
KVM = KVR // P    # 4
DK = D // P       # 16
KT = T // P       # 8 key chunks

_nc_cache = {}

_MARKS = []


def _MARK(nc, label):
    _MARKS.append((label, nc.next_id()))


def build_kernel(use_pad=False):
    import concourse.bacc as bacc
    import concourse.tile as tile
    from concourse import mybir
    from contextlib import ExitStack

    F32 = mybir.dt.float32
    BF = mybir.dt.bfloat16
    F8 = mybir.dt.float8e4
    F8E5 = mybir.dt.float8e5
    AF = mybir.ActivationFunctionType
    DR = mybir.MatmulPerfMode.DoubleRow
    mul = mybir.AluOpType.mult
    add = mybir.AluOpType.add
    sub = mybir.AluOpType.subtract

    nc = bacc.Bacc("TRN2", target_bir_lowering=False, debug=False)

    # ---- DRAM I/O (host-prepared layouts, see _prep_core_inputs) ----
    hq8 = nc.dram_tensor("hq8", [P, DKD, 2, TQ], F8, kind="ExternalInput")
    hk8h = nc.dram_tensor("hk8h", [P, DKD, 2, T], F8, kind="ExternalInput")
    hk8l = nc.dram_tensor("hk8l", [P, DKD, 2, T], F8, kind="ExternalInput")
    wqa8 = nc.dram_tensor("wqa8", [DKD, P, QRM, 2, P], F8, kind="ExternalInput")
    wqb8 = nc.dram_tensor("wqb8", [H, P, QRD, 2, P], F8, kind="ExternalInput")
    wkv8h = nc.dram_tensor("wkv8h", [KVM, P, DKD, 2, P], F8,
                           kind="ExternalInput")
    wkv8l = nc.dram_tensor("wkv8l", [KVM, P, DKD, 2, P], F8,
                           kind="ExternalInput")
    wkvbb = nc.dram_tensor("wkvbb", [P, 2, KVM, P], BF, kind="ExternalInput")
    wob = nc.dram_tensor("wob", [4, P, H, 512], BF, kind="ExternalInput")
    cosq = nc.dram_tensor("cosq", [P, TQ], BF, kind="ExternalInput")
    sinq = nc.dram_tensor("sinq", [P, TQ], BF, kind="ExternalInput")
    c1kv = nc.dram_tensor("c1kv", [P, T], F32, kind="ExternalInput")
    s1kv = nc.dram_tensor("s1kv", [P, T], F32, kind="ExternalInput")
    mask8h = nc.dram_tensor("mask8h", [P, 2, 4, QB], F8E5,
                            kind="ExternalInput")
    gct = nc.dram_tensor("gct", [P, 2], F32, kind="ExternalInput")  # (32a, 32(1-a))
    if use_pad:
        maskp = nc.dram_tensor("maskp", [P, 4, QB], F32, kind="ExternalInput")
    id448 = nc.dram_tensor("id448", [P, 2, P], F8E5, kind="ExternalInput")
    mask8l = nc.dram_tensor("mask8l", [P, 2, 4, QB], F8E5,
                            kind="ExternalInput")
    o_part = nc.dram_tensor("o_part", [TQ, D], F32, kind="ExternalOutput")

    ESC = SCALE / (WS * WS)   # exp scale: scores psum = 1024 * true scores

    with tile.TileContext(nc, pool_alloc_mode="queue") as tc, ExitStack() as top:
        consts = top.enter_context(tc.tile_pool(name="consts", bufs=1))
        # persistent pools first (LIFO pool discipline: transient pools are
        # created after every long-lived one)
        latp = top.enter_context(tc.tile_pool(name="latp", bufs=1))
        kvnb = latp.tile([P, KVM, T], BF)      # 8KB/p unnormalized kv latents
        qn8 = latp.tile([P, QRD, 2, TQ], F8)   # 6KB/p NORMALIZED q latents
        normp = top.enter_context(tc.tile_pool(name="normp", bufs=1))
        rsbq = normp.tile([P, TQ], F32)        # 1/(32*rms_q), bcast partitions
        rsbkv = normp.tile([P, T], F32)        # 1/rms_kv, bcast partitions
        rsbkv_t = normp.tile([P, KT], F32)     # 1/rms_kv, keys on partitions
        wkvbp = top.enter_context(tc.tile_pool(name="wkvbp", bufs=1))
        wb = wkvbp.tile([P, 2, KVM, P], BF)
        kvp = top.enter_context(tc.tile_pool(name="kvp", bufs=1))
        w8 = kvp.tile([P, 2, T], F8)           # [a*kf; (1-a)*kf] x32
        v_b = kvp.tile([P, 4, P], BF)          # bf16 v, key chunks 0-3 (L po)
        v8h = kvp.tile([P, 4, 2, P], F8)       # fp8 v, all 8 chunks (H po)
        # q-up weights: pool reserved up-front (24KB/p) so their DMAs can
        # stream right behind the q-down weights
        wqbp = top.enter_context(tc.tile_pool(name="wqbp", bufs=1))
        qfp = top.enter_context(tc.tile_pool(name="qfp", bufs=1))
        u8 = qfp.tile([P, H, 2, TQ], F8)       # 16KB/p
        outp = top.enter_context(tc.tile_pool(name="outp", bufs=1))
        outT = outp.tile([P, H, TQ], BF)       # 16KB/p

        # ---- transient input pools ----
        dctx = ExitStack()
        wkvap = dctx.enter_context(tc.tile_pool(name="wkvap", bufs=1))
        hkp = dctx.enter_context(tc.tile_pool(name="hkp", bufs=1))
        hkh_sb = hkp.tile([P, DKD, 2, T], F8)  # 16KB/p
        hkl_sb = hkp.tile([P, DKD, 2, T], F8)  # 16KB/p
        # q-path pools last so they release (LIFO) right after q-down
        dctx_q = ExitStack()
        wqap = dctx_q.enter_context(tc.tile_pool(name="wqap", bufs=1))
        hqp = dctx_q.enter_context(tc.tile_pool(name="hqp", bufs=1))
        hq_sb = hqp.tile([P, DKD, 2, TQ], F8)  # 8KB/p

        # ---- input DMAs, one serial device, strict consumption order ----
        # qdown stream
        w_dk = []
        for dk in range(DKD):
            nc.sync.dma_start(hq_sb[:, dk, :, :], hq8[:, dk, :, :])
            w = wqap.tile([P, QRM, 2, P], F8, tag=f"wqa{dk}", name=f"wqa_{dk}")
            nc.sync.dma_start(w[:], wqa8[dk])
            w_dk.append(w)
        # q-up needs cos/sin + wqb right after qdown: stream them next
        cq_t = consts.tile([P, TQ], BF)        # raw blended cos/sin for q
        sq_t = consts.tile([P, TQ], BF)
        nc.sync.dma_start(cq_t[:], cosq[:])
        nc.sync.dma_start(sq_t[:], sinq[:])
        w_hs = []
        for h in range(H):
            w_h = wqbp.tile([P, QRD, 2, P], F8, tag=f"wqb{h}",
                            name=f"wqb_{h}")
            nc.sync.dma_start(w_h[:], wqb8[h])
            w_hs.append(w_h)
        gc_sb = consts.tile([P, 2], F32)
        nc.scalar.dma_start(gc_sb[:], gct[:])
        id_sb = consts.tile([P, 2, P], F8E5)   # 448*I in row 0, zeros row 1
        nc.scalar.dma_start(id_sb[:], id448[:])
        # kv-down stream, matching the m-pair hi->lo consumption order:
        # wvh01, hkh, hkl, wvl01, wvh23, wvl23
        w_kvh, w_kvl = [], []
        for m_ in range(2):
            wvh = wkvap.tile([P, DKD, 2, P], F8, tag=f"wkvh{m_}",
                             name=f"wkvh_{m_}")
            nc.sync.dma_start(wvh[:], wkv8h[m_])
            w_kvh.append(wvh)
        for dk in range(DKD):
            nc.sync.dma_start(hkh_sb[:, dk, :, :], hk8h[:, dk, :, :])
        for dk in range(DKD):
            nc.sync.dma_start(hkl_sb[:, dk, :, :], hk8l[:, dk, :, :])
        for m_ in range(2):
            wvl = wkvap.tile([P, DKD, 2, P], F8, tag=f"wkvl{m_}",
                             name=f"wkvl_{m_}")
            nc.sync.dma_start(wvl[:], wkv8l[m_])
            w_kvl.append(wvl)
        for m_ in range(2, KVM):
            wvh = wkvap.tile([P, DKD, 2, P], F8, tag=f"wkvh{m_}",
                             name=f"wkvh_{m_}")
            nc.sync.dma_start(wvh[:], wkv8h[m_])
            w_kvh.append(wvh)
        for m_ in range(2, KVM):
            wvl = wkvap.tile([P, DKD, 2, P], F8, tag=f"wkvl{m_}",
                             name=f"wkvl_{m_}")
            nc.sync.dma_start(wvl[:], wkv8l[m_])
            w_kvl.append(wvl)
        nc.sync.dma_start(wb[:], wkvbb[:])
        ck_raw = consts.tile([P, T], F32)
        sk_raw = consts.tile([P, T], F32)
        nc.sync.dma_start(ck_raw[:], c1kv[:])
        nc.sync.dma_start(sk_raw[:], s1kv[:])
        ml_sb = consts.tile([P, 2, 4, QB], F8E5)
        mh_sb = consts.tile([P, 2, 4, QB], F8E5)
        nc.sync.dma_start(ml_sb[:], mask8l[:])
        nc.sync.dma_start(mh_sb[:], mask8h[:])
        if use_pad:
            mp_sb = consts.tile([P, 4, QB], F32)
            nc.sync.dma_start(mp_sb[:], maskp[:])
        ones8w = consts.tile([P, 2, P], F8)
        nc.vector.memset(ones8w[:], 1.0)
        ones8 = ones8w[:, :, 0:1]
        onesb = consts.tile([P, 2], BF)
        nc.vector.memset(onesb[:], 1.0)
        eps_sb = consts.tile([P, 1], F32)
        nc.vector.memset(eps_sb[:], 1024.0 * EPS)
        epsn_sb = consts.tile([P, 1], F32)
        nc.vector.memset(epsn_sb[:], EPS)

        _MARK(nc, 'qdown')
        # ---------- q down-proj (fp8 DoubleRow, dk-outer streaming) ----------
        # Two halves of 6 m-chunks so the 6 live psums (+ss) fit in 8 banks;
        # dk-outer order lets the PE consume weight chunks as they stream in.
        with tc.tile_pool(name="sqq", bufs=1) as sqqp, \
             tc.tile_pool(name="qltmp", bufs=1) as qltmp, \
             tc.tile_pool(name="ps_qd", bufs=1, space="PSUM") as psqd, \
             tc.tile_pool(name="ps_ssq", bufs=1, space="PSUM") as psssq:
            ss_q = psssq.tile([P, TQ], F32)
            sq_m = [sqqp.tile([P, 2, TQ], F8, tag=f"sqq{dm}",
                              name=f"sqq_{dm}") for dm in range(QRD)]
            # q_lat parked in SBUF bf16 so psums free up per half (the
            # latents end up fp8 in qn8, so bf16 parking costs nothing)
            qlat = qltmp.tile([P, QRM, TQ], BF)    # 12KB/p, freed after qn8
            for half in range(2):
                ms = list(range(6 * half, 6 * half + 6))
                ps_m = {m: psqd.tile([P, TQ], F32, tag=f"psqd{m % 6}",
                                     name=f"psqd_{m}") for m in ms}
                for dk in range(DKD):
                    for m in ms:
                        for tq in range(2):
                            ts = slice(tq * 256, (tq + 1) * 256)
                            nc.tensor.matmul(
                                ps_m[m][:, ts], w_dk[dk][:, m, :, :],
                                hq_sb[:, dk, :, ts],
                                start=(dk == 0 and tq == 0),
                                stop=(dk == DKD - 1 and tq == 1),
                                perf_mode=DR)
                for m in ms:
                    nc.scalar.activation(sq_m[m // 2][:, m % 2, :],
                                         ps_m[m][:], AF.Square,
                                         scale=1.0 / WS)
                    nc.vector.tensor_copy(qlat[:, m, :], ps_m[m][:])
                # this half's ss_q contribution (accumulated across halves)
                for dm in range(3 * half, 3 * half + 3):
                    for tq in range(2):
                        ts = slice(tq * 256, (tq + 1) * 256)
                        nc.tensor.matmul(
                            ss_q[:, ts], ones8w[:], sq_m[dm][:, :, ts],
                            start=(half == 0 and dm == 0 and tq == 0),
                            stop=(half == 1 and dm == 5 and tq == 1),
                            perf_mode=DR)
            nc.scalar.activation(rsbq[:], ss_q[:], AF.Sqrt,
                                 bias=eps_sb[:], scale=1024.0 / QR)
            nc.vector.reciprocal(rsbq[:], rsbq[:])
            # qn8 = q_lat * rsbq  (pre-normalized latents); DVE/Pool split
            # so the serial tail gating q-up stays short
            for m in range(QRM):
                eng = nc.vector if m < 8 else nc.gpsimd
                eng.tensor_tensor(qn8[:, m // 2, m % 2, :],
                                  qlat[:, m, :], rsbq[:], mul)
        dctx_q.close()

        # ---------- q up-proj (interleaved into kv-down PE stream) ----------
        # Rope per head: Act makes a bf16 psum copy (psb) + the final fp8
        # cast; DVE does the fp8 u0 copy, cos-product, one sin half and the
        # combine (all-bf16 SBUF -> 2x mode); Pool (gpsimd, otherwise idle)
        # takes the other sin half.  The psum frees after the two copies,
        # so the q-up pipeline never waits on the rope tail.
        _MARK(nc, 'qup')

        def q_up(h, rtmpq, psqu):
            ps = psqu.tile([P, TQ], F32, tag="psqu")
            for tq in range(2):
                ts = slice(tq * 256, (tq + 1) * 256)
                for dm in range(QRD):
                    nc.tensor.matmul(ps[:, ts], w_hs[h][:, dm, :, :],
                                     qn8[:, dm, :, ts],
                                     start=(dm == 0), stop=(dm == QRD - 1),
                                     perf_mode=DR)
            nc.scalar.copy(u8[:, h, 0, :], ps[:])
            ta = rtmpq.tile([P, TQ], BF, tag="ta")
            tb = rtmpq.tile([P, TQ], BF, tag="tb")
            # rope products on DVE straight from the psum (partition-shifted
            # reads are only legal from PSUM); the aligned combine runs on
            # Pool (gpsimd: SBUF-only, aligned-only) and writes the fp8
            # u8 slot directly
            nc.vector.tensor_tensor(ta[:], ps[:], cq_t[:], mul)
            nc.vector.tensor_tensor(tb[0:H2, :], ps[H2:P, :],
                                    sq_t[0:H2, :], mul)
            nc.vector.tensor_tensor(tb[H2:P, :], ps[0:H2, :],
                                    sq_t[H2:P, :], mul)
            nc.gpsimd.tensor_tensor(u8[:, h, 1, :], ta[:], tb[:], sub)

        # ---------- kv down-proj: m-pair groups, hi stream then lo ----------
        with tc.tile_pool(name="sqkv", bufs=2) as sqkvp:
          with tc.tile_pool(name="rtmpq", bufs=4) as rtmpq, \
               tc.tile_pool(name="ps_qu", bufs=3, space="PSUM") as psqu:
            qup_h = iter(range(H))
            _MARK(nc, 'kvdown')
            with tc.tile_pool(name="ps_kvd", bufs=1, space="PSUM") as pskvd:
                sq_kv = [sqkvp.tile([P, 2, T], F8, tag=f"sqkv{dm}",
                                    name=f"sqkv_{dm}") for dm in range(2)]
                unit = 0
                for mm in ((0, 1), (2, 3)):
                    mp = 0 if mm == (0, 1) else 1
                    ps_pair = {m: pskvd.tile([P, T], F32, tag=f"pskvd{m % 2}",
                                             name=f"pskvd_{m}") for m in mm}
                    # hi*hi stream (dk-outer so the PE tracks the hkh DMAs),
                    # then the two lo-residual streams (their DMAs trail);
                    # one q-up head woven in every 3 dk-units
                    for si, (hs_, wgetter, st0, st1) in enumerate((
                            (hkh_sb, lambda m: w_kvh[m], True, False),
                            (hkl_sb, lambda m: w_kvh[m], False, False),
                            (hkh_sb, lambda m: w_kvl[m], False, True))):
                        for dk in range(DKD):
                            for m in mm:
                                for sl in range(4):
                                    ts = slice(sl * 256, (sl + 1) * 256)
                                    # psum zero regions are 2KB banks: one
                                    # start/stop per 512-col bank
                                    nc.tensor.matmul(
                                        ps_pair[m][:, ts],
                                        wgetter(m)[:, dk, :, :],
                                        hs_[:, dk, :, ts],
                                        start=(st0 and dk == 0
                                               and sl % 2 == 0),
                                        stop=(st1 and dk == DKD - 1
                                              and sl % 2 == 1),
                                        perf_mode=DR)
                            # pair0-hi runs clean (it's DMA-paced and the
                            # q-up heads would stall on the qn8 tail ahead
                            # of it in the in-order PE queue); heads weave
                            # into the later streams every 2 dk-units
                            if mp == 0 and si == 0:
                                continue
                            unit += 1
                            if unit % 2 == 0:
                                h = next(qup_h, None)
                                if h is not None:
                                    q_up(h, rtmpq, psqu)
                    for m in mm:
                        # latent tn0 halves first: they gate the long chain
                        # k-tn0 -> rope -> w8 -> L attention
                        nc.scalar.copy(kvnb[:, m, 0:512],
                                       ps_pair[m][:, 0:512])
                    for m in mm:
                        nc.scalar.activation(sq_kv[m // 2][:, m % 2, :],
                                             ps_pair[m][:], AF.Square,
                                             scale=1.0 / WS)
                    if mm == (2, 3):
                        # last pair: tn1 halves split Act/DVE so the Act
                        # queue reaches the kvup sqrt sooner
                        nc.scalar.copy(kvnb[:, 2, 512:T],
                                       ps_pair[2][:, 512:T])
                        nc.vector.tensor_copy(kvnb[:, 3, 512:T],
                                              ps_pair[3][:, 512:T])
                    else:
                        for m in mm:
                            nc.scalar.copy(kvnb[:, m, 512:T],
                                           ps_pair[m][:, 512:T])
                for h in qup_h:
                    q_up(h, rtmpq, psqu)

          # ---------- kv up-proj + norm chain ----------
          if True:
            _MARK(nc, 'kvup')
            with tc.tile_pool(name="kfp", bufs=1) as kfp, \
                 tc.tile_pool(name="rtmp", bufs=2) as rtmp, \
                 tc.tile_pool(name="ps_sskv", bufs=1, space="PSUM") as pssskv, \
                 tc.tile_pool(name="ps_sst", bufs=1, space="PSUM") as psst, \
                 tc.tile_pool(name="ps_k", bufs=1, space="PSUM") as psk, \
                 tc.tile_pool(name="ps_v", bufs=1, space="PSUM") as psv:
                ss_kv = pssskv.tile([P, 2, 512], F32)
                ss_t = psst.tile([P, KT], F32)
                kf = kfp.tile([P, T], F32)
                ps_vt = psv.tile([P, KT, P], F32)   # all 8 v chunks, 2 banks
                for dm in range(2):
                    for tq in range(4):
                        ts = slice(tq * 256, (tq + 1) * 256)
                        tnn, to = tq // 2, (tq % 2) * 256
                        nc.tensor.matmul(
                            ss_kv[:, tnn, to:to + 256], ones8w[:],
                            sq_kv[dm][:, :, ts],
                            start=(dm == 0 and to == 0),
                            stop=(dm == 1 and to == 256), perf_mode=DR)
                    for kc in range(KT):
                        ks = slice(kc * P, (kc + 1) * P)
                        nc.tensor.matmul(
                            ss_t[:, kc:kc + 1], sq_kv[dm][:, :, ks],
                            ones8[:],
                            start=(dm == 0 and kc == 0),
                            stop=(dm == 1 and kc == KT - 1), perf_mode=DR)
                for tn in range(2):
                    nc.scalar.activation(rsbkv[:, tn * 512:(tn + 1) * 512],
                                         ss_kv[:, tn, :], AF.Sqrt,
                                         bias=eps_sb[:], scale=1024.0 / KVR)
                nc.scalar.activation(rsbkv_t[:], ss_t[:], AF.Sqrt,
                                     bias=eps_sb[:], scale=1024.0 / KVR)
                recips_done = [False]
                for tn in range(2):
                    ts = slice(tn * 512, (tn + 1) * 512)
                    ps = psk.tile([P, 512], F32, tag="psk")
                    for m in range(KVM):
                        nc.tensor.matmul(ps[:], wb[:, 0, m, :],
                                         kvnb[:, m, ts],
                                         start=(m == 0), stop=(m == KVM - 1))
                    # kf = rope_gate(ps); raw tables (rsbkv folded below)
                    ta = rtmp.tile([P, 512], F32, tag="ta")
                    tb = rtmp.tile([P, 512], F32, tag="tb")
                    nc.vector.tensor_tensor(ta[:], ps[:], ck_raw[:, ts], mul)
                    nc.vector.tensor_tensor(tb[0:H2, :], ps[H2:P, :],
                                            sk_raw[0:H2, ts], mul)
                    nc.vector.tensor_tensor(tb[H2:P, :], ps[0:H2, :],
                                            sk_raw[H2:P, ts], mul)
                    if not recips_done[0]:
                        # recips slot in while the kf combine's inputs are
                        # already computed; sqrt is ready by now
                        recips_done[0] = True
                        nc.vector.reciprocal(rsbkv[:, 0:512],
                                             rsbkv[:, 0:512])
                        nc.vector.reciprocal(rsbkv_t[:], rsbkv_t[:])
                        nc.vector.reciprocal(rsbkv[:, 512:T],
                                             rsbkv[:, 512:T])
                    nc.vector.tensor_tensor(kf[0:H2, ts], ta[0:H2, :],
                                            tb[0:H2, :], sub)
                    nc.vector.tensor_tensor(kf[H2:P, ts], ta[H2:P, :],
                                            tb[H2:P, :], add)
                    # fold rsbkv + write both w8 gate rows for this tn so
                    # the L phase (keys 0-511) unblocks after tn=0
                    nc.vector.tensor_tensor(kf[:, ts], kf[:, ts],
                                            rsbkv[:, ts], mul)
                    nc.vector.tensor_scalar(out=w8[:, 0, ts], in0=kf[:, ts],
                                            scalar1=gc_sb[:, 0:1],
                                            scalar2=None, op0=mul)
                    nc.vector.tensor_scalar(out=w8[:, 1, ts], in0=kf[:, ts],
                                            scalar1=gc_sb[:, 1:2],
                                            scalar2=None, op0=mul)
                    # v up-proj for this tn's key chunks right after the
                    # k chain; Act does the per-partition rms scale
                    for kc in range(4 * tn, 4 * tn + 4):
                        ks = slice(kc * P, (kc + 1) * P)
                        for m in range(KVM):
                            nc.tensor.matmul(ps_vt[:, kc, :],
                                             kvnb[:, m, ks],
                                             wb[:, 1, m, :],
                                             start=(m == 0),
                                             stop=(m == KVM - 1))
                    for kc in range(4 * tn, 4 * tn + 4):
                        nc.scalar.activation(v8h[:, kc // 2, kc % 2, :],
                                             ps_vt[:, kc, :],
                                             AF.Identity,
                                             scale=rsbkv_t[:, kc:kc + 1])
                        if tn == 0:
                            nc.scalar.activation(v_b[:, kc, :],
                                                 ps_vt[:, kc, :],
                                                 AF.Identity,
                                                 scale=rsbkv_t[:, kc:kc + 1])
                # switch the Act table to the exp set here (copy/identity
                # live in both sets) so the L phase's first exp doesn't pay
                # the 1.3us table load
                nc.scalar.activation(epsn_sb[:], eps_sb[:], AF.Exp,
                                     bias=0.0, scale=0.0)
        dctx.close()

        # ---------- attention + o_proj (sw-pipelined) ----------
        _MARK(nc, 'Lphase')
        wop = top.enter_context(tc.tile_pool(name="wop", bufs=1))

        def attn_scores(blk, h, expp, ps_s):
            """Emit scores+mask+exp for head h; returns es tiles.

            Masks are additive on the PE: a DoubleRow inject of the fp8
            {0,-448} mask through the 448*I stationary adds -200704 to the
            psum (exp arg -17.3 -> flushes to exactly 0 in fp8/bf16).
            L keeps bf16 es (early queries: few-key softmax needs the
            precision); H is all-fp8 so po/pr run DoubleRow.
            """
            qs = slice(blk * QB, (blk + 1) * QB)
            es8 = None
            if blk:  # far pairs 0-1, fp8, no causal mask needed
                pss = ps_s.tile([P, 4, QB], F32, tag="pss")
                for kc in range(4):
                    nc.tensor.matmul(pss[:, kc, :],
                                     w8[:, :, kc * P:(kc + 1) * P],
                                     u8[:, h, :, qs], start=True,
                                     stop=True, perf_mode=DR)
                if use_pad:
                    nc.vector.tensor_tensor(pss[:], pss[:], mp_sb[:], add)
                es8 = expp.tile([P, 4, QB], F8, tag="es8")
                nc.scalar.activation(es8[:], pss[:], AF.Exp, bias=0.0,
                                     scale=ESC)
            k0 = 4 if blk else 0
            msk = mh_sb if blk else ml_sb
            pss = ps_s.tile([P, 4, QB], F32, tag="pss")
            for j in range(4):
                kc = k0 + j
                nc.tensor.matmul(pss[:, j, :],
                                 w8[:, :, kc * P:(kc + 1) * P],
                                 u8[:, h, :, qs], start=True,
                                 stop=False, perf_mode=DR)
                nc.tensor.matmul(pss[:, j, :], id_sb[:],
                                 msk[:, :, j, :], start=False, stop=True,
                                 perf_mode=DR)
            esb = expp.tile([P, 4, QB], BF if blk == 0 else F8, tag="esb")
            nc.scalar.activation(esb[:], pss[:], AF.Exp, bias=0.0,
                                 scale=ESC)
            return es8, esb

        def attn_po(blk, h, es, atmp, ps_o):
            """po group then pr group (sequential groups, shared region).
            H (blk=1): everything fp8 DoubleRow (es8 far + fp8 near + v8h).
            L (blk=0): bf16 es x bf16 v (early-query precision)."""
            qs = slice(blk * QB, (blk + 1) * QB)
            es8, esb = es
            po_t = ps_o.tile([P, 2, QB], F32, tag="po")
            po = po_t[:, 0, :]
            pr = po_t[0:1, 1, :]
            if blk:
                for pc, est in ((0, es8), (1, es8), (2, esb), (3, esb)):
                    nc.tensor.matmul(po[:], v8h[:, pc, :, :],
                                     est[:, 2 * (pc % 2):2 * (pc % 2) + 2, :],
                                     start=(pc == 0), stop=(pc == 3),
                                     perf_mode=DR)
                for pc, est in ((0, es8), (1, es8), (2, esb), (3, esb)):
                    nc.tensor.matmul(pr[:], ones8[:],
                                     est[:, 2 * (pc % 2):2 * (pc % 2) + 2, :],
                                     start=(pc == 0), stop=(pc == 3),
                                     perf_mode=DR)
            else:
                for j in range(4):
                    nc.tensor.matmul(po[:], v_b[:, j, :], esb[:, j, :],
                                     start=(j == 0), stop=(j == 3))
                for j in range(4):
                    nc.tensor.matmul(pr[:], onesb[:, 0:1], esb[:, j, :],
                                     start=(j == 0), stop=(j == 3))
            r1r = atmp.tile([1, QB], F32, tag="r1r")
            nc.vector.reciprocal(r1r[:], pr[:])
            rb = atmp.tile([P, QB], F32, tag="rb")
            nc.gpsimd.partition_broadcast(rb[:], r1r[:])
            nc.vector.tensor_tensor(outT[:, h, qs], po[:], rb[:], mul)

        def oproj_tile(i, w_nts, psw, osb, blk):
            qt = blk * 2 + i // 4
            nt = i % 4
            ps = psw.tile([P, 512], F32, tag="psw")
            for h in range(H):
                nc.tensor.matmul(
                    ps[:], outT[:, h, qt * P:(qt + 1) * P],
                    w_nts[nt][:, h, :],
                    start=(h == 0), stop=(h == H - 1))
            ot = osb.tile([P, 512], F32, tag="ot")
            nc.vector.tensor_copy(ot[:], ps[:])
            nc.sync.dma_start(
                o_part[qt * P:(qt + 1) * P, nt * 512:(nt + 1) * 512],
                ot[:])

        with tc.tile_pool(name="expp", bufs=6) as expp, \
             tc.tile_pool(name="atmp", bufs=4) as atmp, \
             tc.tile_pool(name="ps_s", bufs=2, space="PSUM") as ps_s, \
             tc.tile_pool(name="ps_o", bufs=2, space="PSUM") as ps_o:
            w_nts = []
            for nt in range(4):
                w_nt = wop.tile([P, H, 512], BF, tag=f"wo{nt}",
                                name=f"wo_{nt}")
                nc.sync.dma_start(w_nt[:], wob[nt])
                w_nts.append(w_nt)
            # L phase: pure attention, software-pipelined depth 1
            prev = None
            for h in range(H):
                es = attn_scores(0, h, expp, ps_s)
                if prev is not None:
                    attn_po(0, prev[0], prev[1], atmp, ps_o)
                prev = (h, es)
            prevL = prev
            # H phase interleaved with L o_proj tiles, pipelined; the first
            # H scores run ahead of L's last po so the depth-1 software
            # pipeline carries straight across the phase boundary
            _MARK(nc, 'Hphase')
            with tc.tile_pool(name="ps_w", bufs=2, space="PSUM") as ps_w, \
                 tc.tile_pool(name="osb", bufs=3) as osb:
                prev = None
                for h in range(H):
                    es = attn_scores(1, h, expp, ps_s)
                    if h == 0:
                        attn_po(0, prevL[0], prevL[1], atmp, ps_o)
                    if prev is not None:
                        attn_po(1, prev[0], prev[1], atmp, ps_o)
                    prev = (h, es)
                    if h % 2 == 1:
                        oproj_tile(h // 2, w_nts, ps_w, osb, 0)
                attn_po(1, prev[0], prev[1], atmp, ps_o)
                for i in range(7):
                    oproj_tile(i, w_nts, ps_w, osb, 1)
                # last tile as two half-column tiles: the first half's
                # copy+DMA overlap the second half's matmuls, halving the
                # post-PE drain
                for sl in range(2):
                    cs = slice(sl * 256, (sl + 1) * 256)
                    ps = ps_w.tile([P, 256], F32, tag="psw")
                    for h in range(H):
                        nc.tensor.matmul(
                            ps[:], outT[:, h, 3 * P:4 * P],
                            w_nts[3][:, h, cs],
                            start=(h == 0), stop=(h == H - 1))
                    ot = osb.tile([P, 256], F32, tag=f"ot2_{sl}")
                    if sl == 0:
                        nc.vector.tensor_copy(ot[:], ps[:])
                    else:
                        nc.scalar.copy(ot[:], ps[:])
                    nc.sync.dma_start(
                        o_part[3 * P:4 * P,
                               3 * 512 + sl * 256:3 * 512 + (sl + 1) * 256],
                        ot[:])

    _MARK(nc, 'end')
    nc.finalize()
    return nc


def _prep_core_inputs(inputs):
    """Shard + lay out the full inputs for the 8 cores."""
    import ml_dtypes
    F8 = ml_dtypes.float8_e4m3
    F8E5 = ml_dtypes.float8_e5m2
    BF = ml_dtypes.bfloat16
    f32 = np.float32

    hs = np.asarray(inputs["hidden_states"], f32)
    w_qa = np.asarray(inputs["w_qa"], f32)
    w_qb = np.asarray(inputs["w_qb"], f32)
    w_kva = np.asarray(inputs["w_kva"], f32)
    w_kvb = np.asarray(inputs["w_kvb"], f32)
    qn_w = np.asarray(inputs["qn_w"], f32)
    kvn_w = np.asarray(inputs["kvn_w"], f32)
    w_o = np.asarray(inputs["w_o"], f32)
    att_mask = np.asarray(inputs["attention_mask"])
    for bname in ("b_qa", "b_qb", "b_kva", "b_kvb"):
        assert not np.asarray(inputs[bname], f32).any(), \
            "nonzero projection biases not supported"

    a = float(1.0 / (1.0 + np.exp(-f32(inputs["nope_logit"]))))
    g = float(1.0 / (1.0 + np.exp(-f32(inputs["rope_logit"]))))

    w_qb_f = qn_w[:, None] * w_qb
    w_kvb_f = kvn_w[:, None] * w_kvb

    wqa8 = np.ascontiguousarray(
        (w_qa * WS).reshape(DKD, 2, P, QRM, P).transpose(0, 2, 3, 1, 4)
    ).astype(F8)
    wqb8 = np.ascontiguousarray(
        (w_qb_f * WS).reshape(QRD, 2, P, H, P).transpose(3, 2, 0, 1, 4)
    ).astype(F8)
    wkva32 = w_kva * WS
    wkva_hi = wkva32.astype(F8)
    wkva_lo = (wkva32 - wkva_hi.astype(f32)).astype(F8)
    _wkl = lambda w: np.ascontiguousarray(
        w.reshape(DKD, 2, P, KVM, P).transpose(3, 2, 0, 1, 4)).astype(F8)
    wkv8h = _wkl(wkva_hi.astype(f32))
    wkv8l = _wkl(wkva_lo.astype(f32))
    wkvbb = np.ascontiguousarray(
        w_kvb_f.reshape(KVM, P, 2, P).transpose(1, 2, 0, 3)).astype(BF)
    wob = np.ascontiguousarray(
        w_o.reshape(H, P, 4, 512).transpose(2, 1, 0, 3)).astype(BF)
    gct = np.broadcast_to(
        np.array([WS * a, WS * (1.0 - a)], f32), (P, 2)).copy()

    cosb = g * np.asarray(inputs["cos_g"], f32) + (1 - g) * np.asarray(inputs["cos_l"], f32)
    sinb = g * np.asarray(inputs["sin_g"], f32) + (1 - g) * np.asarray(inputs["sin_l"], f32)

    i_p = np.arange(P)
    i_q = np.arange(QB)
    in_maps = []
    for c in range(NCORES):
        b, s = c // 2, c % 2
        blocks = [s, s + 2]
        qcols = np.concatenate([np.arange(bb * QB, (bb + 1) * QB)
                                for bb in blocks])
        hq8 = np.ascontiguousarray(
            hs[b][qcols, :].T.reshape(DKD, 2, P, TQ).transpose(2, 0, 1, 3)
        ).astype(F8)
        hkT = hs[b].T.reshape(DKD, 2, P, T).transpose(2, 0, 1, 3)
        hk_hi = np.ascontiguousarray(hkT).astype(F8)
        hk_lo = np.ascontiguousarray(
            hkT - hk_hi.astype(f32)).astype(F8)
        cb_q = cosb[b][qcols, :].T          # [64, TQ]
        sb_q = sinb[b][qcols, :].T
        cosq = np.ascontiguousarray(
            np.concatenate([cb_q, cb_q], 0)).astype(BF)
        sinq = np.ascontiguousarray(
            np.concatenate([sb_q, -sb_q], 0)).astype(BF)
        cb_k = cosb[b].T                    # [64, T]
        sb_k = sinb[b].T
        c1 = a + (1 - a) * cb_k
        s1 = (1 - a) * sb_k
        c1kv = np.ascontiguousarray(np.concatenate([c1, c1], 0))
        s1kv = np.ascontiguousarray(np.concatenate([s1, s1], 0))
        # masks: key > query  (+ padding): additive {0,-448} fp8, injected
        # through a 448*I DoubleRow stationary (-> -200704 in the psum,
        # exp arg -17.3, flushes to 0).  [P, 2(dbl-row), 4(chunk), QB],
        # second double-row slot zeroed.
        pad_b = (att_mask[b] == 0)
        use_pad = bool(pad_b.any())
        masks = []
        for mi, blk in enumerate(blocks):
            koff = mi * 512  # L-mask covers keys [0:512), H-mask [512:1024)
            key_abs = koff + (np.arange(4)[:, None, None] * P
                              + i_p[None, :, None])        # [4, P, 1]
            q_abs = blk * QB + i_q[None, None, :]          # [1, 1, QB]
            bad = (key_abs > q_abs) | pad_b[key_abs]
            m = (np.where(bad, -448.0, 0.0) + 0.0 * q_abs).transpose(1, 0, 2)
            m2 = np.zeros((P, 2, 4, QB), f32)
            m2[:, 0] = m
            masks.append(np.ascontiguousarray(m2).astype(F8E5))
        id2 = np.zeros((P, 2, P), f32)
        id2[:, 0, :] = 448.0 * np.eye(P)
        imap = {
            "hq8": hq8, "hk8h": hk_hi, "hk8l": hk_lo,
            "wqa8": wqa8, "wqb8": wqb8,
            "wkv8h": wkv8h, "wkv8l": wkv8l, "wkvbb": wkvbb, "wob": wob,
            "cosq": cosq, "sinq": sinq, "c1kv": c1kv, "s1kv": s1kv,
            "mask8l": masks[0], "mask8h": masks[1],
            "gct": gct,
            "id448": id2.astype(F8E5),
        }
        if use_pad:
            # pad-only mask for the H-block far pairs (keys 0-511)
            key_far = (np.arange(4)[:, None, None] * P
                       + i_p[None, :, None])
            mp = np.where(pad_b[key_far], NEG, 0.0) + np.zeros((1, 1, QB))
            imap["maskp"] = np.ascontiguousarray(
                mp.transpose(1, 0, 2)).astype(f32)
        in_maps.append(imap)
    return in_maps


NCORES = 8


def kernel(**inputs):
    use_pad = bool((np.asarray(inputs["attention_mask"]) == 0).any())
    if use_pad not in _nc_cache:
        _nc_cache[use_pad] = build_kernel(use_pad=use_pad)
    nc = _nc_cache[use_pad]

    from concourse.bass_utils import run_bass_kernel_spmd
    in_maps = _prep_core_inputs(inputs)
    res = run_bass_kernel_spmd(nc, in_maps, core_ids=list(range(NCORES)))
    out = np.empty((B, T, D), np.float32)
    for b in range(B):
        for s in range(2):
            r = res.results[2 * b + s]["o_part"]
            for i, blk in enumerate([s, s + 2]):
                out[b, blk * QB:(blk + 1) * QB] = r[i * QB:(i + 1) * QB]
    return out
